# revision 17
# baseline (speedup 1.0000x reference)
"""Trainium2 Bass kernel for a dense transformer block (B=2,T=2048,E=1024,H=16,DH=64,FF=4096).

Sharding: tensor-parallel across 8 NeuronCores — core c computes attention heads
{2c, 2c+1} and FFN columns [512c, 512c+512), returns the transposed partial output
yT = attn_partial^T + ffn_partial^T  [E, B*T] in bf16; the host sums the 8 partials
(the all-reduce), adds the residual x and the output biases bo/b2.

All data is bf16 (matmuls run at the same 1 cycle/row as f32r but halve DMA/SBUF);
PSUM accumulation stays f32, which keeps end-to-end rel err ~2.6e-3.

LayerNorm is folded into the projection matmuls: host precomputes diag(g)@W plus a
33-row augment block (row0 = -g@W coefficient for the mean, row32 = be@W for the
C/sd term); the device computes per-token sum/sumsq with a col-tiled pair of
ones-stationary matmuls (tile_position (0,0)/(0,32) -> the two accumulation chains
run CONCURRENTLY in the PE array), converts to m/sd/r = 1/sd in token-major
[128,4] layout (wide-lane DVE/ACT, no thin reciprocals), and streams [m; sd] as a
K=33 aug matmul into the same PSUM accumulation.

The r scale is applied only to Q and the FFN-u eviction. K and V stay UNSCALED:
the key-side r folds into the softmax exp as a per-partition (per-key) ACT scale
r/8 and bias log r  (ex = exp(s*r_s/8 + log r_s) = r_s * p_s), and the partition
function rides V as an sd-column (Z = sum ex*sd = sum p). 1/Z is computed as
exp(-ln Z) on the ACT engine (DVE reciprocal on [1,512] costs 3.3us; this ~0.9us).

Attention scores (K=64 per head) are ROW-TILED: head0 at PE rows 0-63, head1 at
rows 64-127 run concurrently, doubling score throughput. The relu(u) activations
stay resident in SBUF in bf16 (no DRAM roundtrip). Previous iteration's output
matmuls (W2 first, Wo last) interleave into the attention stream so the tensor
engine never idles while the scalar engine chews exponentials.
"""

import sys
import numpy as np

sys.path.insert(0, "/opt/trn_rl_repo")

from contextlib import ExitStack

import ml_dtypes

import concourse.bacc as bacc
import concourse.bass as bass
import concourse.tile as tile
from concourse import mybir
from concourse.bass_utils import run_bass_kernel_spmd

B, T, E, H, DH, FF = 2, 2048, 1024, 16, 64, 4096
NCORES = 8
NH = H // NCORES      # 2 heads per core
FFC = FF // NCORES    # 512 ffn cols per core
TN = B * T            # 4096 tokens total
TCH = 512             # token chunk
NCH = TN // TCH       # 8 chunks
SC = 128              # s-chunk for attention
NSC = TN // SC        # 32 s-chunks (global)
EK = E // 128         # 8 contraction chunks over E
EPS = 1e-5
LAG = 2               # stats run this many chunks ahead of projections

F32 = mybir.dt.float32
BF16 = mybir.dt.bfloat16
AF = mybir.ActivationFunctionType
OP = mybir.AluOpType

LN8 = float(np.log(0.125))  # fold the 1/sqrt(DH) score scale into r/8


def _bcast(ap, nparts):
    """Partition-broadcast view of a [1, N] DRAM ap -> [nparts, N]."""
    return bass.AP(tensor=ap.tensor, offset=ap.offset, ap=[[0, nparts]] + list(ap.ap[-1:]))


def _tm(row_ap, p=128):
    """Token-major [p, n] view of a [1, p*n] DRAM row slice (token = col*p + part)."""
    return row_ap.rearrange("o (c p) -> (o p) c", p=p)


def _build_device_program(dbg=False):
    nc = bacc.Bacc()

    xT = nc.dram_tensor("xT", [E, TN], BF16, kind="ExternalInput")
    wqkv = nc.dram_tensor("wqkv", [E + 2, 3 * NH * DH], BF16, kind="ExternalInput")
    w1 = nc.dram_tensor("w1", [E + 2, FFC], BF16, kind="ExternalInput")
    w2 = nc.dram_tensor("w2", [FFC, E], BF16, kind="ExternalInput")
    wo = nc.dram_tensor("wo", [NH * DH, E], BF16, kind="ExternalInput")
    ident_in = nc.dram_tensor("ident", [128, 128], BF16, kind="ExternalInput")
    ones_in = nc.dram_tensor("ones1", [128, 1], BF16, kind="ExternalInput")
    yT = nc.dram_tensor("yT", [E, TN], BF16, kind="ExternalOutput")
    if dbg:
        dbg_st = nc.dram_tensor("dbg_st", [4, TN], F32, kind="ExternalOutput")
        dbg_qt = nc.dram_tensor("dbg_qt", [128, TN], BF16, kind="ExternalOutput")
        dbg_kt = nc.dram_tensor("dbg_kt", [128, TN], BF16, kind="ExternalOutput")
        dbg_ot = nc.dram_tensor("dbg_ot", [128, TN], BF16, kind="ExternalOutput")
        dbg_v = nc.dram_tensor("dbg_v", [128, NSC * NH * (DH + 1)], BF16, kind="ExternalOutput")
        dbg_u = nc.dram_tensor("dbg_u", [128, (FFC // 128) * TN], BF16, kind="ExternalOutput")
        dbg_s = nc.dram_tensor("dbg_s", [128, NH * TCH], F32, kind="ExternalOutput")
        dbg_ex = nc.dram_tensor("dbg_ex", [128, NH * TCH], BF16, kind="ExternalOutput")
        dbg_ou = nc.dram_tensor("dbg_ou", [DH + 1, NH * TCH], F32, kind="ExternalOutput")
        dbg_zi = nc.dram_tensor("dbg_zi", [1, NH * TCH], F32, kind="ExternalOutput")

    xTr = xT.rearrange("(c p) t -> p c t", p=128)       # [128, EK, TN]
    yTr = yT.rearrange("(g p) t -> g p t", p=128)       # [8, 128, TN]

    with tile.TileContext(nc) as tc, ExitStack() as top:
        const = top.enter_context(tc.tile_pool(name="const", bufs=1))
        wpool = top.enter_context(tc.tile_pool(name="wpool", bufs=1))
        big = top.enter_context(tc.tile_pool(name="big", bufs=1))
        dpool = top.enter_context(tc.tile_pool(name="dram", bufs=3, space="DRAM"))
        drows = top.enter_context(tc.tile_pool(name="drows", bufs=1, space="DRAM"))

        ident = const.tile([128, 128], BF16)
        ones1 = const.tile([128, 1], BF16)
        nc.sync.dma_start(out=ones1, in_=ones_in[:, :])
        nc.scalar.dma_start(out=ident, in_=ident_in[:, :])
        eps_t = const.tile([128, 1], F32)
        nc.vector.memset(eps_t, EPS)
        ln8_t = const.tile([128, 1], F32)
        nc.vector.memset(ln8_t, LN8)

        # resident weights (loaded on gpsimd queue; needed ~5us in)
        wqkv_sb = wpool.tile([128, EK, 3 * NH * DH], BF16)
        nc.gpsimd.dma_start(out=wqkv_sb, in_=wqkv[0:E, :].rearrange("(c p) d -> p c d", p=128))
        aug_qkv = wpool.tile([97, 3 * NH * DH], BF16)
        nc.vector.memset(aug_qkv, 0.0)
        nc.gpsimd.dma_start(out=aug_qkv[0:1, :], in_=wqkv[E:E + 1, :])
        nc.gpsimd.dma_start(out=aug_qkv[32:33, :], in_=wqkv[E + 1:E + 2, :])
        nc.gpsimd.dma_start(out=aug_qkv[64:65, :], in_=wqkv[E:E + 1, :])
        nc.gpsimd.dma_start(out=aug_qkv[96:97, :], in_=wqkv[E + 1:E + 2, :])
        w1_sb = wpool.tile([128, EK, FFC], BF16)
        nc.gpsimd.dma_start(out=w1_sb, in_=w1[0:E, :].rearrange("(c p) d -> p c d", p=128))
        aug_w1 = wpool.tile([97, FFC], BF16)
        nc.vector.memset(aug_w1, 0.0)
        nc.gpsimd.dma_start(out=aug_w1[0:1, :], in_=w1[E:E + 1, :])
        nc.gpsimd.dma_start(out=aug_w1[32:33, :], in_=w1[E + 1:E + 2, :])
        nc.gpsimd.dma_start(out=aug_w1[64:65, :], in_=w1[E:E + 1, :])
        nc.gpsimd.dma_start(out=aug_w1[96:97, :], in_=w1[E + 1:E + 2, :])
        w2_sb = wpool.tile([128, FFC // 128, E], BF16)
        wo_sb = wpool.tile([128, E], BF16)

        # persistent aug moving buffers (rows 1..31 stay zero)
        aug_bufs = wpool.tile([97, 2, TCH], BF16)
        nc.vector.memset(aug_bufs, 0.0)

        # token-major stats tiles (col k covers tokens [k*128, (k+1)*128))
        r8_all = wpool.tile([128, NSC], F32)    # r/8 (exp scale, includes 1/sqrt(DH))
        rv_all = wpool.tile([128, NSC], F32)    # r (V eviction scale)

        # DRAM stats row (token-ordered [1, TN])
        r_d = drows.tile([1, TN], F32)      # r (Q / u / r_rep broadcast source)

        # resident activations
        QT = big.tile([NH * DH, TN], BF16)
        KT = big.tile([NH * DH, TN], BF16)
        V = big.tile([128, NSC, NH, DH + 1], BF16)   # V natural + sd col
        OT = big.tile([NH * DH, TN], BF16)
        U = big.tile([128, FFC // 128, TN], BF16)    # relu(f@W1+b1)^T resident
        for h in range(NH):
            nc.vector.memset(V[:, :, h, DH:DH + 1], 1.0)

        # ---------------- Phase AB: stats (LAG ahead) + projections ---------
        with ExitStack() as ab, \
             tc.tile_pool(name="xs", bufs=LAG + 2) as xs_pool, \
             tc.tile_pool(name="abwork", bufs=2) as work, \
             tc.tile_pool(name="strows", bufs=2) as strows, \
             tc.tile_pool(name="sttm", bufs=2) as sttm, \
             tc.tile_pool(name="stat_ps", bufs=1, space="PSUM") as stat_ps, \
             tc.tile_pool(name="mm_ps", bufs=3, space="PSUM") as mm_ps, \
             tc.tile_pool(name="vt_ps", bufs=1, space="PSUM") as vt_ps:
            xs_tiles = {}

            def stats_part(t):
                ts0, ts1 = t * TCH, (t + 1) * TCH
                t4 = slice(t * (TCH // 128), (t + 1) * (TCH // 128))
                xs = xs_pool.tile([128, EK, TCH], BF16, tag="xs")
                xs_tiles[t] = xs
                if t == 0:
                    for c in range(EK):
                        eng = nc.sync if c % 2 == 0 else nc.scalar
                        eng.dma_start(out=xs[:, c, :], in_=xTr[:, c, ts0:ts1])
                else:
                    nc.sync.dma_start(out=xs, in_=xTr[:, :, ts0:ts1])
                ps_xa = stat_ps.tile([1, TCH], F32, tag="sxa")
                ps_xb = stat_ps.tile([1, TCH], F32, tag="sxb")
                ps_qa = stat_ps.tile([1, TCH], F32, tag="sqa")
                ps_qb = stat_ps.tile([1, TCH], F32, tag="sqb")
                for c in range(EK):
                    xq = work.tile([128, TCH], BF16, tag="xq", bufs=3)
                    nc.gpsimd.tensor_tensor(out=xq, in0=xs[:, c, :],
                                            in1=xs[:, c, :], op=OP.mult)
                    # row-tiled pairs (rows 0-63 / 64-127 run concurrently)
                    nc.tensor.matmul(ps_xa[0:1, :], ones1[0:64, :], xs[0:64, c, :],
                                     start=(c == 0), stop=(c == EK - 1))
                    nc.tensor.matmul(ps_qb[0:1, :], ones1[64:128, :], xq[64:128, :],
                                     start=(c == 0), stop=(c == EK - 1))
                    nc.tensor.matmul(ps_xb[0:1, :], ones1[64:128, :], xs[64:128, c, :],
                                     start=(c == 0), stop=(c == EK - 1))
                    nc.tensor.matmul(ps_qa[0:1, :], ones1[0:64, :], xq[0:64, :],
                                     start=(c == 0), stop=(c == EK - 1))
                # combine halves + build aug rows / r straight from psum
                aug = aug_bufs[:, t % 2, :]
                mh = strows.tile([1, TCH], F32, tag="mh")
                nc.scalar.activation(out=mh, in_=ps_xa[0:1, :], func=AF.Copy,
                                     scale=1.0 / E)
                mrow = strows.tile([1, TCH], F32, tag="mrow")
                nc.vector.scalar_tensor_tensor(out=mrow, in0=ps_xb[0:1, :],
                                               scalar=1.0 / E, in1=mh,
                                               op0=OP.mult, op1=OP.add)
                nc.scalar.activation(out=aug[0:1, :], in_=mrow, func=AF.Copy)
                nc.scalar.activation(out=aug[64:65, :], in_=mrow, func=AF.Copy)
                qh = strows.tile([1, TCH], F32, tag="qh")
                nc.scalar.activation(out=qh, in_=ps_qa[0:1, :], func=AF.Copy,
                                     scale=1.0 / E)
                sqrow = strows.tile([1, TCH], F32, tag="sqrow")
                nc.vector.scalar_tensor_tensor(out=sqrow, in0=ps_qb[0:1, :],
                                               scalar=1.0 / E, in1=qh,
                                               op0=OP.mult, op1=OP.add)
                nm2 = strows.tile([1, TCH], F32, tag="nm2")
                nc.vector.scalar_tensor_tensor(out=nm2, in0=mrow, scalar=-1.0,
                                               in1=mrow, op0=OP.mult, op1=OP.mult)
                var = strows.tile([1, TCH], F32, tag="var")
                nc.vector.tensor_tensor(out=var, in0=sqrow, in1=nm2, op=OP.add)
                sdrow = strows.tile([1, TCH], F32, tag="sdrow")
                nc.scalar.activation(out=sdrow, in_=var, func=AF.Sqrt,
                                     bias=eps_t[0:1, 0:1])
                nc.scalar.activation(out=aug[32:33, :], in_=sdrow, func=AF.Copy)
                nc.scalar.activation(out=aug[96:97, :], in_=sdrow, func=AF.Copy)
                rrow = strows.tile([1, TCH], F32, tag="rrow")
                nc.vector.reciprocal_approx_fast(out=rrow, in_=sdrow)
                nc.gpsimd.dma_start(out=r_d[0:1, ts0:ts1], in_=rrow)
                if dbg:
                    nc.sync.dma_start(out=dbg_st[2:3, ts0:ts1], in_=rrow)
                    nc.sync.dma_start(out=dbg_st[1:2, ts0:ts1], in_=sdrow)
                    nc.sync.dma_start(out=dbg_st[0:1, ts0:ts1], in_=mrow)
                    vrow_dbg = strows.tile([1, TCH], F32, tag="vdbg")
                    nc.vector.tensor_copy(out=vrow_dbg, in_=var)
                    nc.sync.dma_start(out=dbg_st[3:4, ts0:ts1], in_=vrow_dbg)
                # token-major r views for the exp scale / V eviction scale
                r_tm = sttm.tile([128, TCH // 128], F32, tag="r_tm")
                nc.gpsimd.dma_start(out=r_tm, in_=_tm(r_d[0:1, ts0:ts1]))
                nc.vector.tensor_scalar_mul(r8_all[:, t4], r_tm, 0.125)
                nc.vector.tensor_copy(out=rv_all[:, t4], in_=r_tm)

            def proj_part(t):
                ts0, ts1 = t * TCH, (t + 1) * TCH
                xs = xs_tiles.pop(t)
                aug = aug_bufs[:, t % 2, :]
                r_rep = work.tile([128, TCH], F32, tag="r_rep")
                nc.scalar.dma_start(out=r_rep, in_=_bcast(r_d[0:1, ts0:ts1], 128))

                # 7 output groups (QKV g=0..2, W1 g=3..6). Each group's
                # K=33 aug matmul + eviction is deferred and issued in
                # adjacent pairs on alternating PE row-groups (0-32 / 64-96)
                # so the two aug matmuls run concurrently.
                aug_pend = []

                def flush_aug():
                    while aug_pend:
                        aug_pend.pop(0)()

                def qt_evict(ps):
                    nc.vector.tensor_tensor(out=QT[:, ts0:ts1], in0=ps,
                                            in1=r_rep, op=OP.mult)

                def kt_evict(ps):
                    nc.vector.tensor_copy(out=KT[:, ts0:ts1], in_=ps)

                def v_evict(ps):
                    vt_tmp = work.tile([128, TCH], BF16, tag="vt_tmp")
                    nc.vector.tensor_copy(out=vt_tmp, in_=ps)
                    for j in range(TCH // 128):
                        pvt = vt_ps.tile([128, 128], BF16, tag="pvt")
                        nc.tensor.transpose(
                            pvt, vt_tmp[:, j * 128:(j + 1) * 128], ident)
                        sc = t * (TCH // 128) + j
                        nc.scalar.activation(
                            out=V[:, sc, :, 0:DH],
                            in_=pvt.rearrange("p (h d) -> p h d", h=NH),
                            func=AF.Copy, scale=rv_all[:, sc:sc + 1])

                def u_evict(ps, g):
                    tmp_u = work.tile([128, TCH], F32, tag="tmp_u")
                    nc.vector.tensor_tensor(out=tmp_u, in0=ps, in1=r_rep,
                                            op=OP.mult)
                    nc.scalar.activation(out=U[:, g, ts0:ts1], in_=tmp_u,
                                         func=AF.Relu)

                for gg in range(7):
                    ps = mm_ps.tile([128, TCH], F32, tag="mm")
                    if gg < 3:
                        w_sb, aug_sb, gs = wqkv_sb, aug_qkv, slice(gg * 128, (gg + 1) * 128)
                        ev = qt_evict if gg == 0 else (kt_evict if gg == 1 else v_evict)
                    else:
                        g = gg - 3
                        w_sb, aug_sb, gs = w1_sb, aug_w1, slice(g * 128, (g + 1) * 128)
                        ev = (lambda ps, g=g: u_evict(ps, g))
                    for c in range(EK):
                        nc.tensor.matmul(ps, w_sb[:, c, gs], xs[:, c, :],
                                         start=(c == 0), stop=False)
                    half = slice(0, 33) if (gg % 2 == 0) else slice(64, 97)
                    aug_pend.append(
                        lambda ps=ps, gs=gs, half=half, aug_sb=aug_sb, ev=ev: (
                            nc.tensor.matmul(ps, aug_sb[half, gs], aug[half, :],
                                             start=False, stop=True),
                            ev(ps)))
                    if len(aug_pend) >= 2:
                        flush_aug()
                flush_aug()

            for t in range(NCH + LAG):
                if t >= LAG:
                    proj_part(t - LAG)
                if t < NCH:
                    stats_part(t)

        # deferred CD-only loads (transfer during the projection phase)
        nc.gpsimd.dma_start(out=w2_sb, in_=w2.rearrange("(k p) e -> p k e", p=128))
        nc.gpsimd.dma_start(out=wo_sb, in_=wo[:, :])

        # ---------------- Phase CD: attention + output, per (batch, t-chunk) --
        with ExitStack() as cd, \
             tc.tile_pool(name="expp", bufs=4) as expp, \
             tc.tile_pool(name="cdwork", bufs=2) as cdw, \
             tc.tile_pool(name="zp", bufs=2) as zp, \
             tc.tile_pool(name="s_ps", bufs=2, space="PSUM") as s_ps, \
             tc.tile_pool(name="o_ps", bufs=1, space="PSUM") as o_ps, \
             tc.tile_pool(name="y_ps", bufs=2, space="PSUM") as y_ps:
            nsc = T // SC

            def out_group(ts0, ts1, g):
                # one output tile: y^T[gs, ts] = W2^T u + Wo^T O^T (PSUM-accum)
                ps_y = y_ps.tile([128, TCH], F32, tag="y", name="ps_y")
                gs = slice(g * 128, (g + 1) * 128)
                for k in range(FFC // 128):
                    nc.tensor.matmul(ps_y, w2_sb[:, k, gs], U[:, k, ts0:ts1],
                                     start=(k == 0), stop=False)
                nc.tensor.matmul(ps_y, wo_sb[:, gs], OT[:, ts0:ts1],
                                 start=False, stop=True)
                y_sb = cdw.tile([128, TCH], BF16, tag="y_sb", name="y_sb")
                nc.vector.tensor_copy(out=y_sb, in_=ps_y)
                nc.sync.dma_start(out=yTr[g, :, ts0:ts1], in_=y_sb)

            pending = []  # deferred OUT groups of the previous iteration
            for b in range(B):
                for tq in range(T // TCH):
                    ts0 = b * T + tq * TCH
                    ts1 = ts0 + TCH

                    ps_o = [o_ps.tile([DH + 1, TCH], F32, tag=f"o{h}", name=f"ps_o{h}")
                            for h in range(NH)]
                    exs = {}
                    # scores+exp run one wave ahead of PV; previous iter's OUT
                    # groups interleave so the PE stays dense during exp
                    for sc in range(nsc + 4):
                        if sc < nsc:
                            gsc = b * nsc + sc
                            ps_s = s_ps.tile([128, NH, TCH], F32, tag="s", name="ps_s")
                            for h in range(NH):
                                hs = slice(h * DH, (h + 1) * DH)
                                # row-tiled pair: head0 at PE rows 0-63,
                                # head1 at rows 64-127 run concurrently
                                nc.tensor.matmul(
                                    ps_s[:, h, :], KT[hs, gsc * SC:(gsc + 1) * SC],
                                    QT[hs, ts0:ts1], start=True, stop=True)
                            ex = expp.tile([128, NH, TCH], BF16, tag="ex", name="ex",
                                           bufs=6)
                            nc.scalar.activation(out=ex, in_=ps_s, func=AF.Exp,
                                                 scale=r8_all[:, gsc:gsc + 1])
                            exs[sc] = ex
                            if dbg and b == 0 and tq == 0 and sc == 0:
                                s_sb = cdw.tile([128, NH, TCH], F32, tag="dbgs")
                                nc.vector.tensor_copy(out=s_sb, in_=ps_s)
                                nc.sync.dma_start(
                                    out=dbg_s[:, :],
                                    in_=s_sb.rearrange("p a b -> p (a b)"))
                                nc.sync.dma_start(
                                    out=dbg_ex[:, :],
                                    in_=ex.rearrange("p a b -> p (a b)"))
                        if sc >= 4:
                            psc = sc - 4
                            gpsc = b * nsc + psc
                            ex = exs.pop(psc)
                            for h in range(NH):
                                nc.tensor.matmul(ps_o[h], V[:, gpsc, h, :],
                                                 ex[:, h, :],
                                                 start=(psc == 0),
                                                 stop=(psc == nsc - 1))
                        if pending and sc % 2 == 1:
                            pending.pop(0)()

                    while pending:
                        pending.pop(0)()

                    # evict both accumulators, then normalize via exp(-ln Z)
                    ous = []
                    for h in range(NH):
                        ou = cdw.tile([DH + 1, TCH], F32, tag=f"ou{h}", name="ou")
                        nc.vector.tensor_copy(out=ou, in_=ps_o[h])
                        ous.append(ou)
                    for h in range(NH):
                        ou = ous[h]
                        if dbg and b == 0 and tq == 0:
                            nc.sync.dma_start(
                                out=dbg_ou[:, h * TCH:(h + 1) * TCH], in_=ou)
                        zrow = zp.tile([1, TCH], F32, tag="zrow", name="zrow")
                        nc.scalar.activation(out=zrow, in_=ou[DH:DH + 1, :],
                                             func=AF.Copy)
                        zinv = zp.tile([1, TCH], F32, tag="zinv", name="zinv")
                        nc.vector.reciprocal_approx_fast(out=zinv, in_=zrow)
                        zb = dpool.tile([1, TCH], F32, tag="zb", name="zb")
                        nc.gpsimd.dma_start(out=zb, in_=zinv)
                        if dbg and b == 0 and tq == 0:
                            nc.sync.dma_start(
                                out=dbg_zi[0:1, h * TCH:(h + 1) * TCH], in_=zinv)
                        zrep = zp.tile([DH, TCH], F32, tag="zrep", name="zrep")
                        nc.gpsimd.dma_start(out=zrep, in_=_bcast(zb[0:1, :], DH))
                        nc.vector.tensor_tensor(
                            out=OT[h * DH:(h + 1) * DH, ts0:ts1],
                            in0=ou[0:DH, :], in1=zrep, op=OP.mult)

                    pending = [
                        (lambda g=g, a=ts0, z=ts1: out_group(a, z, g))
                        for g in range(EK)]

            while pending:
                pending.pop(0)()

            if dbg:
                nc.sync.dma_start(out=dbg_qt[:, :], in_=QT)
                nc.sync.dma_start(out=dbg_kt[:, :], in_=KT)
                nc.sync.dma_start(out=dbg_ot[:, :], in_=OT)
                nc.sync.dma_start(out=dbg_v[:, :], in_=V.rearrange("p a b o -> p (a b o)"))
                nc.sync.dma_start(out=dbg_u[:, :], in_=U.rearrange("p a t -> p (a t)"))

    nc.finalize()
    return nc


_CACHE = {}


def _get_program():
    if "nc" not in _CACHE:
        _CACHE["nc"] = _build_device_program()
    return _CACHE["nc"]


def _bf(a):
    return np.ascontiguousarray(np.asarray(a, np.float32)).astype(ml_dtypes.bfloat16)


def _host_prepare(x, Wq, Wk, Wv, Wo, bo, W1, b1, W2, b2, g1, be1, g2, be2):
    xf = np.ascontiguousarray(np.asarray(x, np.float32).reshape(TN, E))
    xT = _bf(xf.T)
    Wq, Wk, Wv = (np.asarray(w, np.float32) for w in (Wq, Wk, Wv))
    Wo, W1, W2 = (np.asarray(w, np.float32) for w in (Wo, W1, W2))
    g1, be1, g2, be2 = (np.asarray(v, np.float32) for v in (g1, be1, g2, be2))
    b1 = np.asarray(b1, np.float32)

    in_maps = []
    for c in range(NCORES):
        hs = [NH * c + i for i in range(NH)]

        def qkv_block(W):
            Wc = W[hs]                                   # [NH, E, DH]
            Wp = (g1[None, :, None] * Wc)                # diag(g1) @ W
            main = np.transpose(Wp, (1, 0, 2)).reshape(E, NH * DH)
            A = np.einsum("e,hed->hd", g1, Wc).reshape(NH * DH)
            C = np.einsum("e,hed->hd", be1, Wc).reshape(NH * DH)
            return np.concatenate([main, -A[None, :], C[None, :]], axis=0)

        wqkv = np.concatenate([qkv_block(Wq), qkv_block(Wk), qkv_block(Wv)], axis=1)

        J = slice(FFC * c, FFC * (c + 1))
        W1c = W1[:, J]
        w1_main = g2[:, None] * W1c
        A1 = g2 @ W1c
        C1 = be2 @ W1c + b1[J]
        w1m = np.concatenate([w1_main, -A1[None, :], C1[None, :]], axis=0)

        in_maps.append({
            "xT": xT,
            "wqkv": _bf(wqkv),
            "w1": _bf(w1m),
            "w2": _bf(W2[J, :]),
            "wo": _bf(Wo[NH * DH * c: NH * DH * (c + 1), :]),
            "ident": _bf(np.eye(128, dtype=np.float32)),
            "ones1": _bf(np.ones((128, 1), np.float32)),
        })
    return xf, in_maps


def _host_finish(x, bo, b2, xf, results):
    acc = xf.copy()
    for res in results:
        acc += np.asarray(res["yT"], np.float32).T
    acc += np.asarray(bo, np.float32)[None, :] + np.asarray(b2, np.float32)[None, :]
    return acc.reshape(np.asarray(x).shape).astype(np.float32)


def kernel(x, Wq, Wk, Wv, Wo, bo, W1, b1, W2, b2, g1, be1, g2, be2, _trace=False):
    nc = _get_program()
    xf, in_maps = _host_prepare(x, Wq, Wk, Wv, Wo, bo, W1, b1, W2, b2, g1, be1, g2, be2)
    out = run_bass_kernel_spmd(nc, in_maps, list(range(NCORES)), trace=_trace)
    result = _host_finish(x, bo, b2, xf, out.results)
    if _trace:
        return result, out
    return result


# revision 18
# speedup vs baseline: 1.0171x; 1.0171x over previous
"""Trainium2 Bass kernel for a dense transformer block (B=2,T=2048,E=1024,H=16,DH=64,FF=4096).

Sharding: tensor-parallel across 8 NeuronCores — core c computes attention heads
{2c, 2c+1} and FFN columns [512c, 512c+512), returns the transposed partial output
yT = attn_partial^T + ffn_partial^T  [E, B*T] in bf16; the host sums the 8 partials
(the all-reduce), adds the residual x and the output biases bo/b2.

All data is bf16 (matmuls run at the same 1 cycle/row as f32r but halve DMA/SBUF);
PSUM accumulation stays f32, which keeps end-to-end rel err ~2.6e-3.

LayerNorm is folded into the projection matmuls: host precomputes diag(g)@W plus a
33-row augment block (row0 = -g@W coefficient for the mean, row32 = be@W for the
C/sd term); the device computes per-token sum/sumsq with a col-tiled pair of
ones-stationary matmuls (tile_position (0,0)/(0,32) -> the two accumulation chains
run CONCURRENTLY in the PE array), converts to m/sd/r = 1/sd in token-major
[128,4] layout (wide-lane DVE/ACT, no thin reciprocals), and streams [m; sd] as a
K=33 aug matmul into the same PSUM accumulation.

The r scale is applied only to Q and the FFN-u eviction. K and V stay UNSCALED:
the key-side r folds into the softmax exp as a per-partition (per-key) ACT scale
r/8 and bias log r  (ex = exp(s*r_s/8 + log r_s) = r_s * p_s), and the partition
function rides V as an sd-column (Z = sum ex*sd = sum p). 1/Z is computed as
exp(-ln Z) on the ACT engine (DVE reciprocal on [1,512] costs 3.3us; this ~0.9us).

Attention scores (K=64 per head) are ROW-TILED: head0 at PE rows 0-63, head1 at
rows 64-127 run concurrently, doubling score throughput. The relu(u) activations
stay resident in SBUF in bf16 (no DRAM roundtrip). Previous iteration's output
matmuls (W2 first, Wo last) interleave into the attention stream so the tensor
engine never idles while the scalar engine chews exponentials.
"""

import sys
import numpy as np

sys.path.insert(0, "/opt/trn_rl_repo")

from contextlib import ExitStack

import ml_dtypes

import concourse.bacc as bacc
import concourse.bass as bass
import concourse.tile as tile
from concourse import mybir
from concourse.bass_utils import run_bass_kernel_spmd

B, T, E, H, DH, FF = 2, 2048, 1024, 16, 64, 4096
NCORES = 8
NH = H // NCORES      # 2 heads per core
FFC = FF // NCORES    # 512 ffn cols per core
TN = B * T            # 4096 tokens total
TCH = 512             # token chunk
NCH = TN // TCH       # 8 chunks
SC = 128              # s-chunk for attention
NSC = TN // SC        # 32 s-chunks (global)
EK = E // 128         # 8 contraction chunks over E
EPS = 1e-5
LAG = 2               # stats run this many chunks ahead of projections

F32 = mybir.dt.float32
BF16 = mybir.dt.bfloat16
AF = mybir.ActivationFunctionType
OP = mybir.AluOpType

LN8 = float(np.log(0.125))  # fold the 1/sqrt(DH) score scale into r/8


def _bcast(ap, nparts):
    """Partition-broadcast view of a [1, N] DRAM ap -> [nparts, N]."""
    return bass.AP(tensor=ap.tensor, offset=ap.offset, ap=[[0, nparts]] + list(ap.ap[-1:]))


def _tm(row_ap, p=128):
    """Token-major [p, n] view of a [1, p*n] DRAM row slice (token = col*p + part)."""
    return row_ap.rearrange("o (c p) -> (o p) c", p=p)


def _build_device_program(dbg=False):
    nc = bacc.Bacc()

    xT = nc.dram_tensor("xT", [E, TN], BF16, kind="ExternalInput")
    wqkv = nc.dram_tensor("wqkv", [E + 2, 3 * NH * DH], BF16, kind="ExternalInput")
    w1 = nc.dram_tensor("w1", [E + 2, FFC], BF16, kind="ExternalInput")
    w2 = nc.dram_tensor("w2", [FFC, E], BF16, kind="ExternalInput")
    wo = nc.dram_tensor("wo", [NH * DH, E], BF16, kind="ExternalInput")
    ident_in = nc.dram_tensor("ident", [128, 128], BF16, kind="ExternalInput")
    ones_in = nc.dram_tensor("ones1", [128, 1], BF16, kind="ExternalInput")
    yT = nc.dram_tensor("yT", [E, TN], BF16, kind="ExternalOutput")
    if dbg:
        dbg_st = nc.dram_tensor("dbg_st", [4, TN], F32, kind="ExternalOutput")
        dbg_qt = nc.dram_tensor("dbg_qt", [128, TN], BF16, kind="ExternalOutput")
        dbg_kt = nc.dram_tensor("dbg_kt", [128, TN], BF16, kind="ExternalOutput")
        dbg_ot = nc.dram_tensor("dbg_ot", [128, TN], BF16, kind="ExternalOutput")
        dbg_v = nc.dram_tensor("dbg_v", [128, NSC * NH * (DH + 1)], BF16, kind="ExternalOutput")
        dbg_u = nc.dram_tensor("dbg_u", [128, (FFC // 128) * TN], BF16, kind="ExternalOutput")
        dbg_s = nc.dram_tensor("dbg_s", [128, NH * TCH], F32, kind="ExternalOutput")
        dbg_ex = nc.dram_tensor("dbg_ex", [128, NH * TCH], BF16, kind="ExternalOutput")
        dbg_ou = nc.dram_tensor("dbg_ou", [DH + 1, NH * TCH], F32, kind="ExternalOutput")
        dbg_zi = nc.dram_tensor("dbg_zi", [1, NH * TCH], F32, kind="ExternalOutput")

    xTr = xT.rearrange("(c p) t -> p c t", p=128)       # [128, EK, TN]
    yTr = yT.rearrange("(g p) t -> g p t", p=128)       # [8, 128, TN]

    with tile.TileContext(nc) as tc, ExitStack() as top:
        const = top.enter_context(tc.tile_pool(name="const", bufs=1))
        wpool = top.enter_context(tc.tile_pool(name="wpool", bufs=1))
        big = top.enter_context(tc.tile_pool(name="big", bufs=1))
        dpool = top.enter_context(tc.tile_pool(name="dram", bufs=3, space="DRAM"))
        drows = top.enter_context(tc.tile_pool(name="drows", bufs=1, space="DRAM"))

        ident = const.tile([128, 128], BF16)
        ones1 = const.tile([128, 1], BF16)
        nc.sync.dma_start(out=ones1, in_=ones_in[:, :])
        nc.scalar.dma_start(out=ident, in_=ident_in[:, :])
        eps_t = const.tile([128, 1], F32)
        nc.vector.memset(eps_t, EPS)
        ln8_t = const.tile([128, 1], F32)
        nc.vector.memset(ln8_t, LN8)

        # resident weights (loaded on gpsimd queue; needed ~5us in)
        wqkv_sb = wpool.tile([128, EK, 3 * NH * DH], BF16)
        nc.gpsimd.dma_start(out=wqkv_sb, in_=wqkv[0:E, :].rearrange("(c p) d -> p c d", p=128))
        aug_qkv = wpool.tile([97, 3 * NH * DH], BF16)
        nc.vector.memset(aug_qkv, 0.0)
        nc.gpsimd.dma_start(out=aug_qkv[0:1, :], in_=wqkv[E:E + 1, :])
        nc.gpsimd.dma_start(out=aug_qkv[32:33, :], in_=wqkv[E + 1:E + 2, :])
        nc.gpsimd.dma_start(out=aug_qkv[64:65, :], in_=wqkv[E:E + 1, :])
        nc.gpsimd.dma_start(out=aug_qkv[96:97, :], in_=wqkv[E + 1:E + 2, :])
        w1_sb = wpool.tile([128, EK, FFC], BF16)
        nc.gpsimd.dma_start(out=w1_sb, in_=w1[0:E, :].rearrange("(c p) d -> p c d", p=128))
        aug_w1 = wpool.tile([97, FFC], BF16)
        nc.vector.memset(aug_w1, 0.0)
        nc.gpsimd.dma_start(out=aug_w1[0:1, :], in_=w1[E:E + 1, :])
        nc.gpsimd.dma_start(out=aug_w1[32:33, :], in_=w1[E + 1:E + 2, :])
        nc.gpsimd.dma_start(out=aug_w1[64:65, :], in_=w1[E:E + 1, :])
        nc.gpsimd.dma_start(out=aug_w1[96:97, :], in_=w1[E + 1:E + 2, :])
        w2_sb = wpool.tile([128, FFC // 128, E], BF16)
        wo_sb = wpool.tile([128, E], BF16)

        # persistent aug moving buffers (rows 1..31 stay zero)
        aug_bufs = wpool.tile([97, 2, TCH], BF16)
        nc.vector.memset(aug_bufs, 0.0)

        # token-major stats tiles (col k covers tokens [k*128, (k+1)*128))
        r8_all = wpool.tile([128, NSC], F32)    # r/8 (exp scale, includes 1/sqrt(DH))
        rv_all = wpool.tile([128, NSC], F32)    # r (V eviction scale)

        # DRAM stats row (token-ordered [1, TN])
        r_d = drows.tile([1, TN], F32)      # r (Q / u / r_rep broadcast source)

        # resident activations
        QT = big.tile([NH * DH, TN], BF16)
        KT = big.tile([NH * DH, TN], BF16)
        V = big.tile([128, NSC, NH, DH + 1], BF16)   # V natural + sd col
        OT = big.tile([NH * DH, TN], BF16)
        U = big.tile([128, FFC // 128, TN], BF16)    # relu(f@W1+b1)^T resident
        for h in range(NH):
            nc.vector.memset(V[:, :, h, DH:DH + 1], 1.0)

        # ---------------- Phase AB: stats (LAG ahead) + projections ---------
        with ExitStack() as ab, \
             tc.tile_pool(name="xs", bufs=LAG + 2) as xs_pool, \
             tc.tile_pool(name="abwork", bufs=2) as work, \
             tc.tile_pool(name="strows", bufs=2) as strows, \
             tc.tile_pool(name="sttm", bufs=2) as sttm, \
             tc.tile_pool(name="stat_ps", bufs=1, space="PSUM") as stat_ps, \
             tc.tile_pool(name="mm_ps", bufs=3, space="PSUM") as mm_ps, \
             tc.tile_pool(name="vt_ps", bufs=1, space="PSUM") as vt_ps:
            xs_tiles = {}

            def stats_part(t):
                ts0, ts1 = t * TCH, (t + 1) * TCH
                t4 = slice(t * (TCH // 128), (t + 1) * (TCH // 128))
                xs = xs_pool.tile([128, EK, TCH], BF16, tag="xs")
                xs_tiles[t] = xs
                if t == 0:
                    for c in range(EK):
                        eng = nc.sync if c % 2 == 0 else nc.scalar
                        eng.dma_start(out=xs[:, c, :], in_=xTr[:, c, ts0:ts1])
                else:
                    nc.sync.dma_start(out=xs, in_=xTr[:, :, ts0:ts1])
                ps_xa = stat_ps.tile([1, TCH], F32, tag="sxa")
                ps_xb = stat_ps.tile([1, TCH], F32, tag="sxb")
                ps_qa = stat_ps.tile([1, TCH], F32, tag="sqa")
                ps_qb = stat_ps.tile([1, TCH], F32, tag="sqb")
                for c in range(EK):
                    xq = work.tile([128, TCH], BF16, tag="xq", bufs=3)
                    nc.scalar.activation(out=xq, in_=xs[:, c, :], func=AF.Square)
                    # row-tiled pairs (rows 0-63 / 64-127 run concurrently)
                    nc.tensor.matmul(ps_xa[0:1, :], ones1[0:64, :], xs[0:64, c, :],
                                     start=(c == 0), stop=(c == EK - 1))
                    nc.tensor.matmul(ps_qb[0:1, :], ones1[64:128, :], xq[64:128, :],
                                     start=(c == 0), stop=(c == EK - 1))
                    nc.tensor.matmul(ps_xb[0:1, :], ones1[64:128, :], xs[64:128, c, :],
                                     start=(c == 0), stop=(c == EK - 1))
                    nc.tensor.matmul(ps_qa[0:1, :], ones1[0:64, :], xq[0:64, :],
                                     start=(c == 0), stop=(c == EK - 1))
                # combine halves + build aug rows / r straight from psum
                aug = aug_bufs[:, t % 2, :]
                mh = strows.tile([1, TCH], F32, tag="mh")
                nc.scalar.activation(out=mh, in_=ps_xa[0:1, :], func=AF.Copy,
                                     scale=1.0 / E)
                mrow = strows.tile([1, TCH], F32, tag="mrow")
                nc.vector.scalar_tensor_tensor(out=mrow, in0=ps_xb[0:1, :],
                                               scalar=1.0 / E, in1=mh,
                                               op0=OP.mult, op1=OP.add)
                nc.scalar.activation(out=aug[0:1, :], in_=mrow, func=AF.Copy)
                nc.scalar.activation(out=aug[64:65, :], in_=mrow, func=AF.Copy)
                qh = strows.tile([1, TCH], F32, tag="qh")
                nc.scalar.activation(out=qh, in_=ps_qa[0:1, :], func=AF.Copy,
                                     scale=1.0 / E)
                sqrow = strows.tile([1, TCH], F32, tag="sqrow")
                nc.vector.scalar_tensor_tensor(out=sqrow, in0=ps_qb[0:1, :],
                                               scalar=1.0 / E, in1=qh,
                                               op0=OP.mult, op1=OP.add)
                nm2 = strows.tile([1, TCH], F32, tag="nm2")
                nc.vector.scalar_tensor_tensor(out=nm2, in0=mrow, scalar=-1.0,
                                               in1=mrow, op0=OP.mult, op1=OP.mult)
                var = strows.tile([1, TCH], F32, tag="var")
                nc.vector.tensor_tensor(out=var, in0=sqrow, in1=nm2, op=OP.add)
                sdrow = strows.tile([1, TCH], F32, tag="sdrow")
                nc.scalar.activation(out=sdrow, in_=var, func=AF.Sqrt,
                                     bias=eps_t[0:1, 0:1])
                nc.scalar.activation(out=aug[32:33, :], in_=sdrow, func=AF.Copy)
                nc.scalar.activation(out=aug[96:97, :], in_=sdrow, func=AF.Copy)
                rrow = strows.tile([1, TCH], F32, tag="rrow")
                nc.vector.reciprocal_approx_fast(out=rrow, in_=sdrow)
                nc.gpsimd.dma_start(out=r_d[0:1, ts0:ts1], in_=rrow)
                if dbg:
                    nc.sync.dma_start(out=dbg_st[2:3, ts0:ts1], in_=rrow)
                    nc.sync.dma_start(out=dbg_st[1:2, ts0:ts1], in_=sdrow)
                    nc.sync.dma_start(out=dbg_st[0:1, ts0:ts1], in_=mrow)
                    vrow_dbg = strows.tile([1, TCH], F32, tag="vdbg")
                    nc.vector.tensor_copy(out=vrow_dbg, in_=var)
                    nc.sync.dma_start(out=dbg_st[3:4, ts0:ts1], in_=vrow_dbg)
                # token-major r views for the exp scale / V eviction scale
                r_tm = sttm.tile([128, TCH // 128], F32, tag="r_tm")
                nc.gpsimd.dma_start(out=r_tm, in_=_tm(r_d[0:1, ts0:ts1]))
                nc.vector.tensor_scalar_mul(r8_all[:, t4], r_tm, 0.125)
                nc.vector.tensor_copy(out=rv_all[:, t4], in_=r_tm)

            def proj_part(t):
                ts0, ts1 = t * TCH, (t + 1) * TCH
                xs = xs_tiles.pop(t)
                aug = aug_bufs[:, t % 2, :]
                r_rep = work.tile([128, TCH], F32, tag="r_rep")
                nc.scalar.dma_start(out=r_rep, in_=_bcast(r_d[0:1, ts0:ts1], 128))

                # 7 output groups (QKV g=0..2, W1 g=3..6). Each group's
                # K=33 aug matmul + eviction is deferred and issued in
                # adjacent pairs on alternating PE row-groups (0-32 / 64-96)
                # so the two aug matmuls run concurrently.
                def qt_evict(ps):
                    nc.vector.tensor_tensor(out=QT[:, ts0:ts1], in0=ps,
                                            in1=r_rep, op=OP.mult)

                def kt_evict(ps):
                    nc.vector.tensor_copy(out=KT[:, ts0:ts1], in_=ps)

                def v_evict(ps):
                    vt_tmp = work.tile([128, TCH], BF16, tag="vt_tmp")
                    nc.vector.tensor_copy(out=vt_tmp, in_=ps)
                    for j in range(TCH // 128):
                        pvt = vt_ps.tile([128, 128], BF16, tag="pvt")
                        nc.tensor.transpose(
                            pvt, vt_tmp[:, j * 128:(j + 1) * 128], ident)
                        sc = t * (TCH // 128) + j
                        nc.scalar.activation(
                            out=V[:, sc, :, 0:DH],
                            in_=pvt.rearrange("p (h d) -> p h d", h=NH),
                            func=AF.Copy, scale=rv_all[:, sc:sc + 1])

                def u_evict(ps, g):
                    tmp_u = work.tile([128, TCH], F32, tag="tmp_u")
                    nc.vector.tensor_tensor(out=tmp_u, in0=ps, in1=r_rep,
                                            op=OP.mult)
                    nc.scalar.activation(out=U[:, g, ts0:ts1], in_=tmp_u,
                                         func=AF.Relu)

                for gg in range(7):
                    ps = mm_ps.tile([128, TCH], F32, tag="mm")
                    if gg < 3:
                        w_sb, aug_sb, gs = wqkv_sb, aug_qkv, slice(gg * 128, (gg + 1) * 128)
                        ev = qt_evict if gg == 0 else (kt_evict if gg == 1 else v_evict)
                    else:
                        g = gg - 3
                        w_sb, aug_sb, gs = w1_sb, aug_w1, slice(g * 128, (g + 1) * 128)
                        ev = (lambda ps, g=g: u_evict(ps, g))
                    for c in range(EK):
                        nc.tensor.matmul(ps, w_sb[:, c, gs], xs[:, c, :],
                                         start=(c == 0), stop=False)
                    nc.tensor.matmul(ps, aug_sb[0:33, gs], aug[0:33, :],
                                     start=False, stop=True)
                    ev(ps)

            for t in range(NCH + LAG):
                if t >= LAG:
                    proj_part(t - LAG)
                if t < NCH:
                    stats_part(t)

        # deferred CD-only loads (transfer during the projection phase)
        nc.gpsimd.dma_start(out=w2_sb, in_=w2.rearrange("(k p) e -> p k e", p=128))
        nc.gpsimd.dma_start(out=wo_sb, in_=wo[:, :])

        # ---------------- Phase CD: attention + output, per (batch, t-chunk) --
        with ExitStack() as cd, \
             tc.tile_pool(name="expp", bufs=4) as expp, \
             tc.tile_pool(name="cdwork", bufs=2) as cdw, \
             tc.tile_pool(name="zp", bufs=2) as zp, \
             tc.tile_pool(name="s_ps", bufs=2, space="PSUM") as s_ps, \
             tc.tile_pool(name="o_ps", bufs=1, space="PSUM") as o_ps, \
             tc.tile_pool(name="y_ps", bufs=2, space="PSUM") as y_ps:
            nsc = T // SC

            def out_group(ts0, ts1, g):
                # one output tile: y^T[gs, ts] = W2^T u + Wo^T O^T (PSUM-accum)
                ps_y = y_ps.tile([128, TCH], F32, tag="y", name="ps_y")
                gs = slice(g * 128, (g + 1) * 128)
                for k in range(FFC // 128):
                    nc.tensor.matmul(ps_y, w2_sb[:, k, gs], U[:, k, ts0:ts1],
                                     start=(k == 0), stop=False)
                nc.tensor.matmul(ps_y, wo_sb[:, gs], OT[:, ts0:ts1],
                                 start=False, stop=True)
                y_sb = cdw.tile([128, TCH], BF16, tag="y_sb", name="y_sb")
                nc.vector.tensor_copy(out=y_sb, in_=ps_y)
                nc.sync.dma_start(out=yTr[g, :, ts0:ts1], in_=y_sb)

            pending = []  # deferred OUT groups of the previous iteration
            for b in range(B):
                for tq in range(T // TCH):
                    ts0 = b * T + tq * TCH
                    ts1 = ts0 + TCH

                    ps_o = [o_ps.tile([DH + 1, TCH], F32, tag=f"o{h}", name=f"ps_o{h}")
                            for h in range(NH)]
                    exs = {}
                    # scores+exp run one wave ahead of PV; previous iter's OUT
                    # groups interleave so the PE stays dense during exp
                    for sc in range(nsc + 4):
                        if sc < nsc:
                            gsc = b * nsc + sc
                            ps_s = s_ps.tile([128, NH, TCH], F32, tag="s", name="ps_s")
                            for h in range(NH):
                                hs = slice(h * DH, (h + 1) * DH)
                                # row-tiled pair: head0 at PE rows 0-63,
                                # head1 at rows 64-127 run concurrently
                                nc.tensor.matmul(
                                    ps_s[:, h, :], KT[hs, gsc * SC:(gsc + 1) * SC],
                                    QT[hs, ts0:ts1], start=True, stop=True)
                            ex = expp.tile([128, NH, TCH], BF16, tag="ex", name="ex",
                                           bufs=6)
                            nc.scalar.activation(out=ex, in_=ps_s, func=AF.Exp,
                                                 scale=r8_all[:, gsc:gsc + 1])
                            exs[sc] = ex
                            if dbg and b == 0 and tq == 0 and sc == 0:
                                s_sb = cdw.tile([128, NH, TCH], F32, tag="dbgs")
                                nc.vector.tensor_copy(out=s_sb, in_=ps_s)
                                nc.sync.dma_start(
                                    out=dbg_s[:, :],
                                    in_=s_sb.rearrange("p a b -> p (a b)"))
                                nc.sync.dma_start(
                                    out=dbg_ex[:, :],
                                    in_=ex.rearrange("p a b -> p (a b)"))
                        if sc >= 4:
                            psc = sc - 4
                            gpsc = b * nsc + psc
                            ex = exs.pop(psc)
                            for h in range(NH):
                                nc.tensor.matmul(ps_o[h], V[:, gpsc, h, :],
                                                 ex[:, h, :],
                                                 start=(psc == 0),
                                                 stop=(psc == nsc - 1))
                        if pending and sc >= 2 and sc % 2 == 0:
                            pending.pop(0)()

                    while pending:
                        pending.pop(0)()

                    # evict both accumulators, then normalize via exp(-ln Z)
                    ous = []
                    for h in range(NH):
                        ou = cdw.tile([DH + 1, TCH], F32, tag=f"ou{h}", name="ou")
                        nc.vector.tensor_copy(out=ou, in_=ps_o[h])
                        ous.append(ou)
                    for h in range(NH):
                        ou = ous[h]
                        if dbg and b == 0 and tq == 0:
                            nc.sync.dma_start(
                                out=dbg_ou[:, h * TCH:(h + 1) * TCH], in_=ou)
                        zrow = zp.tile([1, TCH], F32, tag="zrow", name="zrow")
                        nc.scalar.activation(out=zrow, in_=ou[DH:DH + 1, :],
                                             func=AF.Copy)
                        zinv = zp.tile([1, TCH], F32, tag="zinv", name="zinv")
                        nc.vector.reciprocal_approx_fast(out=zinv, in_=zrow)
                        zb = dpool.tile([1, TCH], F32, tag="zb", name="zb")
                        nc.gpsimd.dma_start(out=zb, in_=zinv)
                        if dbg and b == 0 and tq == 0:
                            nc.sync.dma_start(
                                out=dbg_zi[0:1, h * TCH:(h + 1) * TCH], in_=zinv)
                        zrep = zp.tile([DH, TCH], F32, tag="zrep", name="zrep")
                        nc.gpsimd.dma_start(out=zrep, in_=_bcast(zb[0:1, :], DH))
                        nc.vector.tensor_tensor(
                            out=OT[h * DH:(h + 1) * DH, ts0:ts1],
                            in0=ou[0:DH, :], in1=zrep, op=OP.mult)

                    pending = [
                        (lambda g=g, a=ts0, z=ts1: out_group(a, z, g))
                        for g in range(EK)]

            while pending:
                pending.pop(0)()

            if dbg:
                nc.sync.dma_start(out=dbg_qt[:, :], in_=QT)
                nc.sync.dma_start(out=dbg_kt[:, :], in_=KT)
                nc.sync.dma_start(out=dbg_ot[:, :], in_=OT)
                nc.sync.dma_start(out=dbg_v[:, :], in_=V.rearrange("p a b o -> p (a b o)"))
                nc.sync.dma_start(out=dbg_u[:, :], in_=U.rearrange("p a t -> p (a t)"))

    nc.finalize()
    return nc


_CACHE = {}


def _get_program():
    if "nc" not in _CACHE:
        _CACHE["nc"] = _build_device_program()
    return _CACHE["nc"]


def _bf(a):
    return np.ascontiguousarray(np.asarray(a, np.float32)).astype(ml_dtypes.bfloat16)


def _host_prepare(x, Wq, Wk, Wv, Wo, bo, W1, b1, W2, b2, g1, be1, g2, be2):
    xf = np.ascontiguousarray(np.asarray(x, np.float32).reshape(TN, E))
    xT = _bf(xf.T)
    Wq, Wk, Wv = (np.asarray(w, np.float32) for w in (Wq, Wk, Wv))
    Wo, W1, W2 = (np.asarray(w, np.float32) for w in (Wo, W1, W2))
    g1, be1, g2, be2 = (np.asarray(v, np.float32) for v in (g1, be1, g2, be2))
    b1 = np.asarray(b1, np.float32)

    in_maps = []
    for c in range(NCORES):
        hs = [NH * c + i for i in range(NH)]

        def qkv_block(W):
            Wc = W[hs]                                   # [NH, E, DH]
            Wp = (g1[None, :, None] * Wc)                # diag(g1) @ W
            main = np.transpose(Wp, (1, 0, 2)).reshape(E, NH * DH)
            A = np.einsum("e,hed->hd", g1, Wc).reshape(NH * DH)
            C = np.einsum("e,hed->hd", be1, Wc).reshape(NH * DH)
            return np.concatenate([main, -A[None, :], C[None, :]], axis=0)

        wqkv = np.concatenate([qkv_block(Wq), qkv_block(Wk), qkv_block(Wv)], axis=1)

        J = slice(FFC * c, FFC * (c + 1))
        W1c = W1[:, J]
        w1_main = g2[:, None] * W1c
        A1 = g2 @ W1c
        C1 = be2 @ W1c + b1[J]
        w1m = np.concatenate([w1_main, -A1[None, :], C1[None, :]], axis=0)

        in_maps.append({
            "xT": xT,
            "wqkv": _bf(wqkv),
            "w1": _bf(w1m),
            "w2": _bf(W2[J, :]),
            "wo": _bf(Wo[NH * DH * c: NH * DH * (c + 1), :]),
            "ident": _bf(np.eye(128, dtype=np.float32)),
            "ones1": _bf(np.ones((128, 1), np.float32)),
        })
    return xf, in_maps


def _host_finish(x, bo, b2, xf, results):
    acc = xf.copy()
    for res in results:
        acc += np.asarray(res["yT"], np.float32).T
    acc += np.asarray(bo, np.float32)[None, :] + np.asarray(b2, np.float32)[None, :]
    return acc.reshape(np.asarray(x).shape).astype(np.float32)


def kernel(x, Wq, Wk, Wv, Wo, bo, W1, b1, W2, b2, g1, be1, g2, be2, _trace=False):
    nc = _get_program()
    xf, in_maps = _host_prepare(x, Wq, Wk, Wv, Wo, bo, W1, b1, W2, b2, g1, be1, g2, be2)
    out = run_bass_kernel_spmd(nc, in_maps, list(range(NCORES)), trace=_trace)
    result = _host_finish(x, bo, b2, xf, out.results)
    if _trace:
        return result, out
    return result


# revision 19
# speedup vs baseline: 1.0538x; 1.0361x over previous
"""Trainium2 Bass kernel for a dense transformer block (B=2,T=2048,E=1024,H=16,DH=64,FF=4096).

Sharding: tensor-parallel across 8 NeuronCores — core c computes attention heads
{2c, 2c+1} and FFN columns [512c, 512c+512), returns the transposed partial output
yT = attn_partial^T + ffn_partial^T  [E, B*T] in bf16; the host sums the 8 partials
(the all-reduce), adds the residual x and the output biases bo/b2.

All data is bf16 (matmuls run at the same 1 cycle/row as f32r but halve DMA/SBUF);
PSUM accumulation stays f32, which keeps end-to-end rel err ~2.6e-3.

LayerNorm is folded into the projection matmuls: host precomputes diag(g)@W plus a
33-row augment block (row0 = -g@W coefficient for the mean, row32 = be@W for the
C/sd term); the device computes per-token sum/sumsq with a col-tiled pair of
ones-stationary matmuls (tile_position (0,0)/(0,32) -> the two accumulation chains
run CONCURRENTLY in the PE array), converts to m/sd/r = 1/sd in token-major
[128,4] layout (wide-lane DVE/ACT, no thin reciprocals), and streams [m; sd] as a
K=33 aug matmul into the same PSUM accumulation.

The r scale is applied only to Q and the FFN-u eviction. K and V stay UNSCALED:
the key-side r folds into the softmax exp as a per-partition (per-key) ACT scale
r/8 and bias log r  (ex = exp(s*r_s/8 + log r_s) = r_s * p_s), and the partition
function rides V as an sd-column (Z = sum ex*sd = sum p). 1/Z is computed as
exp(-ln Z) on the ACT engine (DVE reciprocal on [1,512] costs 3.3us; this ~0.9us).

Attention scores (K=64 per head) are ROW-TILED: head0 at PE rows 0-63, head1 at
rows 64-127 run concurrently, doubling score throughput. The relu(u) activations
stay resident in SBUF in bf16 (no DRAM roundtrip). Previous iteration's output
matmuls (W2 first, Wo last) interleave into the attention stream so the tensor
engine never idles while the scalar engine chews exponentials.
"""

import sys
import numpy as np

sys.path.insert(0, "/opt/trn_rl_repo")

from contextlib import ExitStack

import ml_dtypes

import concourse.bacc as bacc
import concourse.bass as bass
import concourse.tile as tile
from concourse import mybir
from concourse.bass_utils import run_bass_kernel_spmd

B, T, E, H, DH, FF = 2, 2048, 1024, 16, 64, 4096
NCORES = 8
NH = H // NCORES      # 2 heads per core
FFC = FF // NCORES    # 512 ffn cols per core
TN = B * T            # 4096 tokens total
TCH = 512             # token chunk
NCH = TN // TCH       # 8 chunks
SC = 128              # s-chunk for attention
NSC = TN // SC        # 32 s-chunks (global)
EK = E // 128         # 8 contraction chunks over E
EPS = 1e-5
LAG = 2               # stats run this many chunks ahead of projections

F32 = mybir.dt.float32
BF16 = mybir.dt.bfloat16
AF = mybir.ActivationFunctionType
OP = mybir.AluOpType

LN8 = float(np.log(0.125))  # fold the 1/sqrt(DH) score scale into r/8


def _bcast(ap, nparts):
    """Partition-broadcast view of a [1, N] DRAM ap -> [nparts, N]."""
    return bass.AP(tensor=ap.tensor, offset=ap.offset, ap=[[0, nparts]] + list(ap.ap[-1:]))


def _tm(row_ap, p=128):
    """Token-major [p, n] view of a [1, p*n] DRAM row slice (token = col*p + part)."""
    return row_ap.rearrange("o (c p) -> (o p) c", p=p)


def _build_device_program(dbg=False):
    nc = bacc.Bacc()

    xT = nc.dram_tensor("xT", [E, TN], BF16, kind="ExternalInput")
    wqkv = nc.dram_tensor("wqkv", [E + 2, 3 * NH * DH], BF16, kind="ExternalInput")
    w1 = nc.dram_tensor("w1", [E + 2, FFC], BF16, kind="ExternalInput")
    w2 = nc.dram_tensor("w2", [FFC, E], BF16, kind="ExternalInput")
    wo = nc.dram_tensor("wo", [NH * DH, E], BF16, kind="ExternalInput")
    ident_in = nc.dram_tensor("ident", [128, 128], BF16, kind="ExternalInput")
    ones_in = nc.dram_tensor("ones1", [128, 1], BF16, kind="ExternalInput")
    yT = nc.dram_tensor("yT", [E, TN], BF16, kind="ExternalOutput")
    if dbg:
        dbg_st = nc.dram_tensor("dbg_st", [4, TN], F32, kind="ExternalOutput")
        dbg_qt = nc.dram_tensor("dbg_qt", [128, TN], BF16, kind="ExternalOutput")
        dbg_kt = nc.dram_tensor("dbg_kt", [128, TN], BF16, kind="ExternalOutput")
        dbg_ot = nc.dram_tensor("dbg_ot", [128, TN], BF16, kind="ExternalOutput")
        dbg_v = nc.dram_tensor("dbg_v", [128, NSC * NH * (DH + 1)], BF16, kind="ExternalOutput")
        dbg_u = nc.dram_tensor("dbg_u", [128, (FFC // 128) * TN], BF16, kind="ExternalOutput")
        dbg_s = nc.dram_tensor("dbg_s", [128, NH * TCH], F32, kind="ExternalOutput")
        dbg_ex = nc.dram_tensor("dbg_ex", [128, NH * TCH], BF16, kind="ExternalOutput")
        dbg_ou = nc.dram_tensor("dbg_ou", [DH + 1, NH * TCH], F32, kind="ExternalOutput")
        dbg_zi = nc.dram_tensor("dbg_zi", [1, NH * TCH], F32, kind="ExternalOutput")

    xTr = xT.rearrange("(c p) t -> p c t", p=128)       # [128, EK, TN]
    yTr = yT.rearrange("(g p) t -> g p t", p=128)       # [8, 128, TN]

    with tile.TileContext(nc) as tc, ExitStack() as top:
        const = top.enter_context(tc.tile_pool(name="const", bufs=1))
        wpool = top.enter_context(tc.tile_pool(name="wpool", bufs=1))
        big = top.enter_context(tc.tile_pool(name="big", bufs=1))
        dpool = top.enter_context(tc.tile_pool(name="dram", bufs=3, space="DRAM"))
        drows = top.enter_context(tc.tile_pool(name="drows", bufs=1, space="DRAM"))

        ident = const.tile([128, 128], BF16)
        ones1 = const.tile([128, 1], BF16)
        nc.sync.dma_start(out=ones1, in_=ones_in[:, :])
        nc.scalar.dma_start(out=ident, in_=ident_in[:, :])
        eps_t = const.tile([128, 1], F32)
        nc.vector.memset(eps_t, EPS)
        ln8_t = const.tile([128, 1], F32)
        nc.vector.memset(ln8_t, LN8)

        # resident weights (loaded on gpsimd queue; needed ~5us in)
        wqkv_sb = wpool.tile([128, EK, 3 * NH * DH], BF16)
        nc.gpsimd.dma_start(out=wqkv_sb, in_=wqkv[0:E, :].rearrange("(c p) d -> p c d", p=128))
        aug_qkv = wpool.tile([97, 3 * NH * DH], BF16)
        nc.vector.memset(aug_qkv, 0.0)
        nc.gpsimd.dma_start(out=aug_qkv[0:1, :], in_=wqkv[E:E + 1, :])
        nc.gpsimd.dma_start(out=aug_qkv[32:33, :], in_=wqkv[E + 1:E + 2, :])
        nc.gpsimd.dma_start(out=aug_qkv[64:65, :], in_=wqkv[E:E + 1, :])
        nc.gpsimd.dma_start(out=aug_qkv[96:97, :], in_=wqkv[E + 1:E + 2, :])
        w1_sb = wpool.tile([128, EK, FFC], BF16)
        nc.gpsimd.dma_start(out=w1_sb, in_=w1[0:E, :].rearrange("(c p) d -> p c d", p=128))
        aug_w1 = wpool.tile([97, FFC], BF16)
        nc.vector.memset(aug_w1, 0.0)
        nc.gpsimd.dma_start(out=aug_w1[0:1, :], in_=w1[E:E + 1, :])
        nc.gpsimd.dma_start(out=aug_w1[32:33, :], in_=w1[E + 1:E + 2, :])
        nc.gpsimd.dma_start(out=aug_w1[64:65, :], in_=w1[E:E + 1, :])
        nc.gpsimd.dma_start(out=aug_w1[96:97, :], in_=w1[E + 1:E + 2, :])
        w2_sb = wpool.tile([128, FFC // 128, E], BF16)
        wo_sb = wpool.tile([128, E], BF16)

        # persistent aug moving buffers (rows 1..31 stay zero)
        aug_bufs = wpool.tile([97, 2, TCH], BF16)
        nc.vector.memset(aug_bufs, 0.0)

        # token-major stats tiles (col k covers tokens [k*128, (k+1)*128))
        r8_all = wpool.tile([128, NSC], F32)    # r/8 (exp scale, includes 1/sqrt(DH))
        rv_all = wpool.tile([128, NSC], F32)    # r (V eviction scale)

        # DRAM stats row (token-ordered [1, TN])
        r_d = drows.tile([1, TN], F32)      # r (Q / u / r_rep broadcast source)

        # resident activations
        QT = big.tile([NH * DH, TN], BF16)
        KT = big.tile([NH * DH, TN], BF16)
        V = big.tile([128, NSC, NH, DH + 1], BF16)   # V natural + sd col
        OT = big.tile([NH * DH, TN], BF16)
        U = big.tile([128, FFC // 128, TN], BF16)    # relu(f@W1+b1)^T resident
        for h in range(NH):
            nc.vector.memset(V[:, :, h, DH:DH + 1], 1.0)

        # ---------------- Phase AB: stats (LAG ahead) + projections ---------
        with ExitStack() as ab, \
             tc.tile_pool(name="xs", bufs=LAG + 2) as xs_pool, \
             tc.tile_pool(name="abwork", bufs=2) as work, \
             tc.tile_pool(name="strows", bufs=2) as strows, \
             tc.tile_pool(name="sttm", bufs=2) as sttm, \
             tc.tile_pool(name="stat_ps", bufs=1, space="PSUM") as stat_ps, \
             tc.tile_pool(name="mm_ps", bufs=3, space="PSUM") as mm_ps, \
             tc.tile_pool(name="vt_ps", bufs=2, space="PSUM") as vt_ps:
            xs_tiles = {}

            def stats_part(t):
                ts0, ts1 = t * TCH, (t + 1) * TCH
                t4 = slice(t * (TCH // 128), (t + 1) * (TCH // 128))
                xs = xs_pool.tile([128, EK, TCH], BF16, tag="xs")
                xs_tiles[t] = xs
                if t == 0:
                    for c in range(EK):
                        eng = nc.sync if c % 2 == 0 else nc.scalar
                        eng.dma_start(out=xs[:, c, :], in_=xTr[:, c, ts0:ts1])
                else:
                    nc.sync.dma_start(out=xs, in_=xTr[:, :, ts0:ts1])
                ps = stat_ps.tile([1, TCH], F32, tag="stx")
                ps_q = stat_ps.tile([33, TCH], F32, tag="stq")
                for c in range(EK):
                    xq = work.tile([128, TCH], BF16, tag="xq", bufs=3)
                    nc.scalar.activation(out=xq, in_=xs[:, c, :], func=AF.Square)
                    nc.tensor.matmul(ps[0:1, :], ones1, xs[:, c, :],
                                     start=(c == 0), stop=(c == EK - 1))
                    nc.tensor.matmul(ps_q[32:33, :], ones1, xq,
                                     start=(c == 0), stop=(c == EK - 1))
                # aug rows + r straight from the psum rows
                aug = aug_bufs[:, t % 2, :]
                nc.scalar.activation(out=aug[0:1, :], in_=ps[0:1, :], func=AF.Copy,
                                     scale=1.0 / E)
                mrow = strows.tile([1, TCH], F32, tag="mrow")
                nc.scalar.activation(out=mrow, in_=ps[0:1, :], func=AF.Copy,
                                     scale=1.0 / E)
                nm2 = strows.tile([1, TCH], F32, tag="nm2")
                nc.vector.scalar_tensor_tensor(out=nm2, in0=mrow, scalar=-1.0,
                                               in1=mrow, op0=OP.mult, op1=OP.mult)
                var = strows.tile([1, TCH], F32, tag="var")
                nc.vector.scalar_tensor_tensor(out=var, in0=ps_q[32:33, :],
                                               scalar=1.0 / E, in1=nm2,
                                               op0=OP.mult, op1=OP.add)
                sdrow = strows.tile([1, TCH], F32, tag="sdrow")
                nc.scalar.activation(out=sdrow, in_=var, func=AF.Sqrt,
                                     bias=eps_t[0:1, 0:1])
                nc.scalar.activation(out=aug[32:33, :], in_=sdrow, func=AF.Copy)
                rrow = strows.tile([1, TCH], F32, tag="rrow")
                nc.vector.reciprocal_approx_fast(out=rrow, in_=sdrow)
                nc.gpsimd.dma_start(out=r_d[0:1, ts0:ts1], in_=rrow)
                if dbg:
                    nc.sync.dma_start(out=dbg_st[2:3, ts0:ts1], in_=rrow)
                    nc.sync.dma_start(out=dbg_st[1:2, ts0:ts1], in_=sdrow)
                    nc.sync.dma_start(out=dbg_st[0:1, ts0:ts1], in_=mrow)
                    vrow_dbg = strows.tile([1, TCH], F32, tag="vdbg")
                    nc.vector.tensor_copy(out=vrow_dbg, in_=var)
                    nc.sync.dma_start(out=dbg_st[3:4, ts0:ts1], in_=vrow_dbg)
                # token-major r views for the exp scale / V eviction scale
                r_tm = sttm.tile([128, TCH // 128], F32, tag="r_tm")
                nc.gpsimd.dma_start(out=r_tm, in_=_tm(r_d[0:1, ts0:ts1]))
                nc.vector.tensor_scalar_mul(r8_all[:, t4], r_tm, 0.125)
                nc.vector.tensor_copy(out=rv_all[:, t4], in_=r_tm)

            def proj_part(t):
                ts0, ts1 = t * TCH, (t + 1) * TCH
                xs = xs_tiles.pop(t)
                aug = aug_bufs[:, t % 2, :]
                r_rep = work.tile([128, TCH], F32, tag="r_rep")
                nc.scalar.dma_start(out=r_rep, in_=_bcast(r_d[0:1, ts0:ts1], 128))

                # 7 output groups (QKV g=0..2, W1 g=3..6). Each group's
                # K=33 aug matmul + eviction is deferred and issued in
                # adjacent pairs on alternating PE row-groups (0-32 / 64-96)
                # so the two aug matmuls run concurrently.
                def qt_evict(ps):
                    nc.vector.tensor_tensor(out=QT[:, ts0:ts1], in0=ps,
                                            in1=r_rep, op=OP.mult)

                def kt_evict(ps):
                    nc.vector.tensor_copy(out=KT[:, ts0:ts1], in_=ps)

                def v_evict(ps):
                    vt_tmp = work.tile([128, TCH], BF16, tag="vt_tmp")
                    nc.vector.tensor_copy(out=vt_tmp, in_=ps)
                    for j in range(TCH // 128):
                        pvt = vt_ps.tile([128, 128], BF16, tag="pvt")
                        nc.tensor.transpose(
                            pvt, vt_tmp[:, j * 128:(j + 1) * 128], ident)
                        sc = t * (TCH // 128) + j
                        nc.scalar.activation(
                            out=V[:, sc, :, 0:DH],
                            in_=pvt.rearrange("p (h d) -> p h d", h=NH),
                            func=AF.Copy, scale=rv_all[:, sc:sc + 1])

                def u_evict(ps, g):
                    tmp_u = work.tile([128, TCH], F32, tag="tmp_u")
                    nc.vector.tensor_tensor(out=tmp_u, in0=ps, in1=r_rep,
                                            op=OP.mult)
                    nc.scalar.activation(out=U[:, g, ts0:ts1], in_=tmp_u,
                                         func=AF.Relu)

                for gg in range(7):
                    ps = mm_ps.tile([128, TCH], F32, tag="mm")
                    if gg < 3:
                        w_sb, aug_sb, gs = wqkv_sb, aug_qkv, slice(gg * 128, (gg + 1) * 128)
                        ev = qt_evict if gg == 0 else (kt_evict if gg == 1 else v_evict)
                    else:
                        g = gg - 3
                        w_sb, aug_sb, gs = w1_sb, aug_w1, slice(g * 128, (g + 1) * 128)
                        ev = (lambda ps, g=g: u_evict(ps, g))
                    for c in range(EK):
                        nc.tensor.matmul(ps, w_sb[:, c, gs], xs[:, c, :],
                                         start=(c == 0), stop=False)
                    nc.tensor.matmul(ps, aug_sb[0:33, gs], aug[0:33, :],
                                     start=False, stop=True)
                    ev(ps)

            for t in range(NCH + LAG):
                if t >= LAG:
                    proj_part(t - LAG)
                if t < NCH:
                    stats_part(t)

        # deferred CD-only loads (transfer during the projection phase)
        nc.gpsimd.dma_start(out=w2_sb, in_=w2.rearrange("(k p) e -> p k e", p=128))
        nc.gpsimd.dma_start(out=wo_sb, in_=wo[:, :])

        # ---------------- Phase CD: attention + output, per (batch, t-chunk) --
        with ExitStack() as cd, \
             tc.tile_pool(name="expp", bufs=4) as expp, \
             tc.tile_pool(name="cdwork", bufs=2) as cdw, \
             tc.tile_pool(name="zp", bufs=2) as zp, \
             tc.tile_pool(name="s_ps", bufs=2, space="PSUM") as s_ps, \
             tc.tile_pool(name="o_ps", bufs=1, space="PSUM") as o_ps, \
             tc.tile_pool(name="y_ps", bufs=2, space="PSUM") as y_ps:
            nsc = T // SC

            def out_group(ts0, ts1, g):
                # one output tile: y^T[gs, ts] = W2^T u + Wo^T O^T (PSUM-accum)
                ps_y = y_ps.tile([128, TCH], F32, tag="y", name="ps_y")
                gs = slice(g * 128, (g + 1) * 128)
                for k in range(FFC // 128):
                    nc.tensor.matmul(ps_y, w2_sb[:, k, gs], U[:, k, ts0:ts1],
                                     start=(k == 0), stop=False)
                nc.tensor.matmul(ps_y, wo_sb[:, gs], OT[:, ts0:ts1],
                                 start=False, stop=True)
                y_sb = cdw.tile([128, TCH], BF16, tag="y_sb", name="y_sb")
                nc.vector.tensor_copy(out=y_sb, in_=ps_y)
                nc.sync.dma_start(out=yTr[g, :, ts0:ts1], in_=y_sb)

            pending = []  # deferred OUT groups of the previous iteration
            for b in range(B):
                for tq in range(T // TCH):
                    ts0 = b * T + tq * TCH
                    ts1 = ts0 + TCH

                    ps_o = [o_ps.tile([DH + 1, TCH], F32, tag=f"o{h}", name=f"ps_o{h}")
                            for h in range(NH)]
                    exs = {}
                    # scores+exp run one wave ahead of PV; previous iter's OUT
                    # groups interleave so the PE stays dense during exp
                    for sc in range(nsc + 4):
                        if sc < nsc:
                            gsc = b * nsc + sc
                            ps_s = s_ps.tile([128, NH, TCH], F32, tag="s", name="ps_s")
                            for h in range(NH):
                                hs = slice(h * DH, (h + 1) * DH)
                                # row-tiled pair: head0 at PE rows 0-63,
                                # head1 at rows 64-127 run concurrently
                                nc.tensor.matmul(
                                    ps_s[:, h, :], KT[hs, gsc * SC:(gsc + 1) * SC],
                                    QT[hs, ts0:ts1], start=True, stop=True)
                            ex = expp.tile([128, NH, TCH], BF16, tag="ex", name="ex",
                                           bufs=6)
                            nc.scalar.activation(out=ex, in_=ps_s, func=AF.Exp,
                                                 scale=r8_all[:, gsc:gsc + 1])
                            exs[sc] = ex
                            if dbg and b == 0 and tq == 0 and sc == 0:
                                s_sb = cdw.tile([128, NH, TCH], F32, tag="dbgs")
                                nc.vector.tensor_copy(out=s_sb, in_=ps_s)
                                nc.sync.dma_start(
                                    out=dbg_s[:, :],
                                    in_=s_sb.rearrange("p a b -> p (a b)"))
                                nc.sync.dma_start(
                                    out=dbg_ex[:, :],
                                    in_=ex.rearrange("p a b -> p (a b)"))
                        if sc >= 4:
                            psc = sc - 4
                            gpsc = b * nsc + psc
                            ex = exs.pop(psc)
                            for h in range(NH):
                                nc.tensor.matmul(ps_o[h], V[:, gpsc, h, :],
                                                 ex[:, h, :],
                                                 start=(psc == 0),
                                                 stop=(psc == nsc - 1))
                        if pending and sc >= 2 and sc % 2 == 0:
                            pending.pop(0)()

                    while pending:
                        pending.pop(0)()

                    # evict both accumulators, then normalize via exp(-ln Z)
                    ous = []
                    for h in range(NH):
                        ou = cdw.tile([DH + 1, TCH], F32, tag=f"ou{h}", name="ou")
                        nc.vector.tensor_copy(out=ou, in_=ps_o[h])
                        ous.append(ou)
                    for h in range(NH):
                        ou = ous[h]
                        if dbg and b == 0 and tq == 0:
                            nc.sync.dma_start(
                                out=dbg_ou[:, h * TCH:(h + 1) * TCH], in_=ou)
                        zrow = zp.tile([1, TCH], F32, tag="zrow", name="zrow")
                        nc.scalar.activation(out=zrow, in_=ou[DH:DH + 1, :],
                                             func=AF.Copy)
                        zinv = zp.tile([1, TCH], F32, tag="zinv", name="zinv")
                        nc.vector.reciprocal_approx_fast(out=zinv, in_=zrow)
                        zb = dpool.tile([1, TCH], F32, tag="zb", name="zb")
                        nc.gpsimd.dma_start(out=zb, in_=zinv)
                        if dbg and b == 0 and tq == 0:
                            nc.sync.dma_start(
                                out=dbg_zi[0:1, h * TCH:(h + 1) * TCH], in_=zinv)
                        zrep = zp.tile([DH, TCH], F32, tag="zrep", name="zrep")
                        nc.gpsimd.dma_start(out=zrep, in_=_bcast(zb[0:1, :], DH))
                        nc.vector.tensor_tensor(
                            out=OT[h * DH:(h + 1) * DH, ts0:ts1],
                            in0=ou[0:DH, :], in1=zrep, op=OP.mult)

                    pending = [
                        (lambda g=g, a=ts0, z=ts1: out_group(a, z, g))
                        for g in range(EK)]

            while pending:
                pending.pop(0)()

            if dbg:
                nc.sync.dma_start(out=dbg_qt[:, :], in_=QT)
                nc.sync.dma_start(out=dbg_kt[:, :], in_=KT)
                nc.sync.dma_start(out=dbg_ot[:, :], in_=OT)
                nc.sync.dma_start(out=dbg_v[:, :], in_=V.rearrange("p a b o -> p (a b o)"))
                nc.sync.dma_start(out=dbg_u[:, :], in_=U.rearrange("p a t -> p (a t)"))

    nc.finalize()
    return nc


_CACHE = {}


def _get_program():
    if "nc" not in _CACHE:
        _CACHE["nc"] = _build_device_program()
    return _CACHE["nc"]


def _bf(a):
    return np.ascontiguousarray(np.asarray(a, np.float32)).astype(ml_dtypes.bfloat16)


def _host_prepare(x, Wq, Wk, Wv, Wo, bo, W1, b1, W2, b2, g1, be1, g2, be2):
    xf = np.ascontiguousarray(np.asarray(x, np.float32).reshape(TN, E))
    xT = _bf(xf.T)
    Wq, Wk, Wv = (np.asarray(w, np.float32) for w in (Wq, Wk, Wv))
    Wo, W1, W2 = (np.asarray(w, np.float32) for w in (Wo, W1, W2))
    g1, be1, g2, be2 = (np.asarray(v, np.float32) for v in (g1, be1, g2, be2))
    b1 = np.asarray(b1, np.float32)

    in_maps = []
    for c in range(NCORES):
        hs = [NH * c + i for i in range(NH)]

        def qkv_block(W):
            Wc = W[hs]                                   # [NH, E, DH]
            Wp = (g1[None, :, None] * Wc)                # diag(g1) @ W
            main = np.transpose(Wp, (1, 0, 2)).reshape(E, NH * DH)
            A = np.einsum("e,hed->hd", g1, Wc).reshape(NH * DH)
            C = np.einsum("e,hed->hd", be1, Wc).reshape(NH * DH)
            return np.concatenate([main, -A[None, :], C[None, :]], axis=0)

        wqkv = np.concatenate([qkv_block(Wq), qkv_block(Wk), qkv_block(Wv)], axis=1)

        J = slice(FFC * c, FFC * (c + 1))
        W1c = W1[:, J]
        w1_main = g2[:, None] * W1c
        A1 = g2 @ W1c
        C1 = be2 @ W1c + b1[J]
        w1m = np.concatenate([w1_main, -A1[None, :], C1[None, :]], axis=0)

        in_maps.append({
            "xT": xT,
            "wqkv": _bf(wqkv),
            "w1": _bf(w1m),
            "w2": _bf(W2[J, :]),
            "wo": _bf(Wo[NH * DH * c: NH * DH * (c + 1), :]),
            "ident": _bf(np.eye(128, dtype=np.float32)),
            "ones1": _bf(np.ones((128, 1), np.float32)),
        })
    return xf, in_maps


def _host_finish(x, bo, b2, xf, results):
    acc = xf.copy()
    for res in results:
        acc += np.asarray(res["yT"], np.float32).T
    acc += np.asarray(bo, np.float32)[None, :] + np.asarray(b2, np.float32)[None, :]
    return acc.reshape(np.asarray(x).shape).astype(np.float32)


def kernel(x, Wq, Wk, Wv, Wo, bo, W1, b1, W2, b2, g1, be1, g2, be2, _trace=False):
    nc = _get_program()
    xf, in_maps = _host_prepare(x, Wq, Wk, Wv, Wo, bo, W1, b1, W2, b2, g1, be1, g2, be2)
    out = run_bass_kernel_spmd(nc, in_maps, list(range(NCORES)), trace=_trace)
    result = _host_finish(x, bo, b2, xf, out.results)
    if _trace:
        return result, out
    return result


# revision 20
# speedup vs baseline: 1.0644x; 1.0101x over previous
"""Trainium2 Bass kernel for a dense transformer block (B=2,T=2048,E=1024,H=16,DH=64,FF=4096).

Sharding: tensor-parallel across 8 NeuronCores — core c computes attention heads
{2c, 2c+1} and FFN columns [512c, 512c+512), returns the transposed partial output
yT = attn_partial^T + ffn_partial^T  [E, B*T] in bf16; the host sums the 8 partials
(the all-reduce), adds the residual x and the output biases bo/b2.

All data is bf16 (matmuls run at the same 1 cycle/row as f32r but halve DMA/SBUF);
PSUM accumulation stays f32, which keeps end-to-end rel err ~2.6e-3.

LayerNorm is folded into the projection matmuls: host precomputes diag(g)@W plus a
33-row augment block (row0 = -g@W coefficient for the mean, row32 = be@W for the
C/sd term); the device computes per-token sum/sumsq with a col-tiled pair of
ones-stationary matmuls (tile_position (0,0)/(0,32) -> the two accumulation chains
run CONCURRENTLY in the PE array), converts to m/sd/r = 1/sd in token-major
[128,4] layout (wide-lane DVE/ACT, no thin reciprocals), and streams [m; sd] as a
K=33 aug matmul into the same PSUM accumulation.

The r scale is applied only to Q and the FFN-u eviction. K and V stay UNSCALED:
the key-side r folds into the softmax exp as a per-partition (per-key) ACT scale
r/8 and bias log r  (ex = exp(s*r_s/8 + log r_s) = r_s * p_s), and the partition
function rides V as an sd-column (Z = sum ex*sd = sum p). 1/Z is computed as
exp(-ln Z) on the ACT engine (DVE reciprocal on [1,512] costs 3.3us; this ~0.9us).

Attention scores (K=64 per head) are ROW-TILED: head0 at PE rows 0-63, head1 at
rows 64-127 run concurrently, doubling score throughput. The relu(u) activations
stay resident in SBUF in bf16 (no DRAM roundtrip). Previous iteration's output
matmuls (W2 first, Wo last) interleave into the attention stream so the tensor
engine never idles while the scalar engine chews exponentials.
"""

import sys
import numpy as np

sys.path.insert(0, "/opt/trn_rl_repo")

from contextlib import ExitStack

import ml_dtypes

import concourse.bacc as bacc
import concourse.bass as bass
import concourse.tile as tile
from concourse import mybir
from concourse.bass_utils import run_bass_kernel_spmd

B, T, E, H, DH, FF = 2, 2048, 1024, 16, 64, 4096
NCORES = 8
NH = H // NCORES      # 2 heads per core
FFC = FF // NCORES    # 512 ffn cols per core
TN = B * T            # 4096 tokens total
TCH = 512             # token chunk
NCH = TN // TCH       # 8 chunks
SC = 128              # s-chunk for attention
NSC = TN // SC        # 32 s-chunks (global)
EK = E // 128         # 8 contraction chunks over E
EPS = 1e-5
LAG = 2               # stats run this many chunks ahead of projections

F32 = mybir.dt.float32
BF16 = mybir.dt.bfloat16
AF = mybir.ActivationFunctionType
OP = mybir.AluOpType

LN8 = float(np.log(0.125))  # fold the 1/sqrt(DH) score scale into r/8


def _bcast(ap, nparts):
    """Partition-broadcast view of a [1, N] DRAM ap -> [nparts, N]."""
    return bass.AP(tensor=ap.tensor, offset=ap.offset, ap=[[0, nparts]] + list(ap.ap[-1:]))


def _tm(row_ap, p=128):
    """Token-major [p, n] view of a [1, p*n] DRAM row slice (token = col*p + part)."""
    return row_ap.rearrange("o (c p) -> (o p) c", p=p)


def _build_device_program(dbg=False):
    nc = bacc.Bacc()

    xT = nc.dram_tensor("xT", [E, TN], BF16, kind="ExternalInput")
    wqkv = nc.dram_tensor("wqkv", [E + 2, 3 * NH * DH], BF16, kind="ExternalInput")
    w1 = nc.dram_tensor("w1", [E + 2, FFC], BF16, kind="ExternalInput")
    w2 = nc.dram_tensor("w2", [FFC, E], BF16, kind="ExternalInput")
    wo = nc.dram_tensor("wo", [NH * DH, E], BF16, kind="ExternalInput")
    ident_in = nc.dram_tensor("ident", [128, 128], BF16, kind="ExternalInput")
    ones_in = nc.dram_tensor("ones1", [128, 1], BF16, kind="ExternalInput")
    yT = nc.dram_tensor("yT", [E, TN], BF16, kind="ExternalOutput")
    if dbg:
        dbg_st = nc.dram_tensor("dbg_st", [4, TN], F32, kind="ExternalOutput")
        dbg_qt = nc.dram_tensor("dbg_qt", [128, TN], BF16, kind="ExternalOutput")
        dbg_kt = nc.dram_tensor("dbg_kt", [128, TN], BF16, kind="ExternalOutput")
        dbg_ot = nc.dram_tensor("dbg_ot", [128, TN], BF16, kind="ExternalOutput")
        dbg_v = nc.dram_tensor("dbg_v", [128, NSC * NH * (DH + 1)], BF16, kind="ExternalOutput")
        dbg_u = nc.dram_tensor("dbg_u", [128, (FFC // 128) * TN], BF16, kind="ExternalOutput")
        dbg_s = nc.dram_tensor("dbg_s", [128, NH * TCH], F32, kind="ExternalOutput")
        dbg_ex = nc.dram_tensor("dbg_ex", [128, NH * TCH], BF16, kind="ExternalOutput")
        dbg_ou = nc.dram_tensor("dbg_ou", [DH + 1, NH * TCH], F32, kind="ExternalOutput")
        dbg_zi = nc.dram_tensor("dbg_zi", [1, NH * TCH], F32, kind="ExternalOutput")

    xTr = xT.rearrange("(c p) t -> p c t", p=128)       # [128, EK, TN]
    yTr = yT.rearrange("(g p) t -> g p t", p=128)       # [8, 128, TN]

    with tile.TileContext(nc) as tc, ExitStack() as top:
        const = top.enter_context(tc.tile_pool(name="const", bufs=1))
        wpool = top.enter_context(tc.tile_pool(name="wpool", bufs=1))
        big = top.enter_context(tc.tile_pool(name="big", bufs=1))
        dpool = top.enter_context(tc.tile_pool(name="dram", bufs=3, space="DRAM"))
        drows = top.enter_context(tc.tile_pool(name="drows", bufs=1, space="DRAM"))

        ident = const.tile([128, 128], BF16)
        ones1 = const.tile([128, 1], BF16)
        nc.sync.dma_start(out=ones1, in_=ones_in[:, :])
        nc.scalar.dma_start(out=ident, in_=ident_in[:, :])
        eps_t = const.tile([128, 1], F32)
        nc.vector.memset(eps_t, EPS)
        ln8_t = const.tile([128, 1], F32)
        nc.vector.memset(ln8_t, LN8)

        # resident weights (loaded on gpsimd queue; needed ~5us in)
        wqkv_sb = wpool.tile([128, EK, 3 * NH * DH], BF16)
        nc.gpsimd.dma_start(out=wqkv_sb, in_=wqkv[0:E, :].rearrange("(c p) d -> p c d", p=128))
        aug_qkv = wpool.tile([97, 3 * NH * DH], BF16)
        nc.vector.memset(aug_qkv, 0.0)
        nc.gpsimd.dma_start(out=aug_qkv[0:1, :], in_=wqkv[E:E + 1, :])
        nc.gpsimd.dma_start(out=aug_qkv[32:33, :], in_=wqkv[E + 1:E + 2, :])
        nc.gpsimd.dma_start(out=aug_qkv[64:65, :], in_=wqkv[E:E + 1, :])
        nc.gpsimd.dma_start(out=aug_qkv[96:97, :], in_=wqkv[E + 1:E + 2, :])
        w1_sb = wpool.tile([128, EK, FFC], BF16)
        nc.gpsimd.dma_start(out=w1_sb, in_=w1[0:E, :].rearrange("(c p) d -> p c d", p=128))
        aug_w1 = wpool.tile([97, FFC], BF16)
        nc.vector.memset(aug_w1, 0.0)
        nc.gpsimd.dma_start(out=aug_w1[0:1, :], in_=w1[E:E + 1, :])
        nc.gpsimd.dma_start(out=aug_w1[32:33, :], in_=w1[E + 1:E + 2, :])
        nc.gpsimd.dma_start(out=aug_w1[64:65, :], in_=w1[E:E + 1, :])
        nc.gpsimd.dma_start(out=aug_w1[96:97, :], in_=w1[E + 1:E + 2, :])
        w2_sb = wpool.tile([128, FFC // 128, E], BF16)
        wo_sb = wpool.tile([128, E], BF16)

        # persistent aug moving buffers (rows 1..31 stay zero)
        aug_bufs = wpool.tile([97, 2, TCH], BF16)
        nc.vector.memset(aug_bufs, 0.0)

        # token-major stats tiles (col k covers tokens [k*128, (k+1)*128))
        r8_all = wpool.tile([128, NSC], F32)    # r/8 (exp scale, includes 1/sqrt(DH))
        rv_all = wpool.tile([128, NSC], F32)    # r (V eviction scale)

        # DRAM stats row (token-ordered [1, TN])
        r_d = drows.tile([1, TN], F32)      # r (Q / u / r_rep broadcast source)

        # resident activations
        QT = big.tile([NH * DH, TN], BF16)
        KT = big.tile([NH * DH, TN], BF16)
        V = big.tile([128, NSC, NH, DH + 1], BF16)   # V natural + sd col
        OT = big.tile([NH * DH, TN], BF16)
        U = big.tile([128, FFC // 128, TN], BF16)    # relu(f@W1+b1)^T resident
        for h in range(NH):
            nc.vector.memset(V[:, :, h, DH:DH + 1], 1.0)

        # ---------------- Phase AB: stats (LAG ahead) + projections ---------
        with ExitStack() as ab, \
             tc.tile_pool(name="xs", bufs=LAG + 2) as xs_pool, \
             tc.tile_pool(name="abwork", bufs=2) as work, \
             tc.tile_pool(name="strows", bufs=2) as strows, \
             tc.tile_pool(name="sttm", bufs=2) as sttm, \
             tc.tile_pool(name="stat_ps", bufs=1, space="PSUM") as stat_ps, \
             tc.tile_pool(name="mm_ps", bufs=3, space="PSUM") as mm_ps, \
             tc.tile_pool(name="vt_ps", bufs=2, space="PSUM") as vt_ps:
            xs_tiles = {}

            def stats_part(t):
                ts0, ts1 = t * TCH, (t + 1) * TCH
                t4 = slice(t * (TCH // 128), (t + 1) * (TCH // 128))
                xs = xs_pool.tile([128, EK, TCH], BF16, tag="xs")
                xs_tiles[t] = xs
                if t == 0:
                    for c in range(EK):
                        eng = nc.sync if c % 2 == 0 else nc.scalar
                        eng.dma_start(out=xs[:, c, :], in_=xTr[:, c, ts0:ts1])
                else:
                    nc.sync.dma_start(out=xs, in_=xTr[:, :, ts0:ts1])
                ps = stat_ps.tile([1, TCH], F32, tag="stx")
                ps_q = stat_ps.tile([33, TCH], F32, tag="stq")
                for c in range(EK):
                    xq = work.tile([128, TCH], BF16, tag="xq", bufs=3)
                    nc.scalar.activation(out=xq, in_=xs[:, c, :], func=AF.Square)
                    nc.tensor.matmul(ps[0:1, :], ones1, xs[:, c, :],
                                     start=(c == 0), stop=(c == EK - 1))
                    nc.tensor.matmul(ps_q[32:33, :], ones1, xq,
                                     start=(c == 0), stop=(c == EK - 1))
                # aug rows + r straight from the psum rows
                aug = aug_bufs[:, t % 2, :]
                nc.scalar.activation(out=aug[0:1, :], in_=ps[0:1, :], func=AF.Copy,
                                     scale=1.0 / E)
                mrow = strows.tile([1, TCH], F32, tag="mrow")
                nc.scalar.activation(out=mrow, in_=ps[0:1, :], func=AF.Copy,
                                     scale=1.0 / E)
                nm2 = strows.tile([1, TCH], F32, tag="nm2")
                nc.vector.scalar_tensor_tensor(out=nm2, in0=mrow, scalar=-1.0,
                                               in1=mrow, op0=OP.mult, op1=OP.mult)
                var = strows.tile([1, TCH], F32, tag="var")
                nc.vector.scalar_tensor_tensor(out=var, in0=ps_q[32:33, :],
                                               scalar=1.0 / E, in1=nm2,
                                               op0=OP.mult, op1=OP.add)
                sdrow = strows.tile([1, TCH], F32, tag="sdrow")
                nc.scalar.activation(out=sdrow, in_=var, func=AF.Sqrt,
                                     bias=eps_t[0:1, 0:1])
                nc.scalar.activation(out=aug[32:33, :], in_=sdrow, func=AF.Copy)
                rrow = strows.tile([1, TCH], F32, tag="rrow")
                nc.vector.reciprocal_approx_fast(out=rrow, in_=sdrow)
                nc.gpsimd.dma_start(out=r_d[0:1, ts0:ts1], in_=rrow)
                if dbg:
                    nc.sync.dma_start(out=dbg_st[2:3, ts0:ts1], in_=rrow)
                    nc.sync.dma_start(out=dbg_st[1:2, ts0:ts1], in_=sdrow)
                    nc.sync.dma_start(out=dbg_st[0:1, ts0:ts1], in_=mrow)
                    vrow_dbg = strows.tile([1, TCH], F32, tag="vdbg")
                    nc.vector.tensor_copy(out=vrow_dbg, in_=var)
                    nc.sync.dma_start(out=dbg_st[3:4, ts0:ts1], in_=vrow_dbg)
                # token-major r views for the exp scale / V eviction scale
                r_tm = sttm.tile([128, TCH // 128], F32, tag="r_tm")
                nc.gpsimd.dma_start(out=r_tm, in_=_tm(r_d[0:1, ts0:ts1]))
                nc.vector.tensor_scalar_mul(r8_all[:, t4], r_tm, 0.125)
                nc.vector.tensor_copy(out=rv_all[:, t4], in_=r_tm)

            def proj_part(t):
                ts0, ts1 = t * TCH, (t + 1) * TCH
                xs = xs_tiles.pop(t)
                aug = aug_bufs[:, t % 2, :]
                r_rep = work.tile([128, TCH], F32, tag="r_rep")
                nc.scalar.dma_start(out=r_rep, in_=_bcast(r_d[0:1, ts0:ts1], 128))

                # 7 output groups (QKV g=0..2, W1 g=3..6). Each group's
                # K=33 aug matmul + eviction is deferred and issued in
                # adjacent pairs on alternating PE row-groups (0-32 / 64-96)
                # so the two aug matmuls run concurrently.
                def qt_evict(ps):
                    nc.vector.tensor_tensor(out=QT[:, ts0:ts1], in0=ps,
                                            in1=r_rep, op=OP.mult)

                def kt_evict(ps):
                    nc.vector.tensor_copy(out=KT[:, ts0:ts1], in_=ps)

                def v_evict(ps):
                    vt_tmp = work.tile([128, TCH], BF16, tag="vt_tmp")
                    nc.vector.tensor_copy(out=vt_tmp, in_=ps)
                    for j in range(TCH // 128):
                        pvt = vt_ps.tile([128, 128], BF16, tag="pvt")
                        nc.tensor.transpose(
                            pvt, vt_tmp[:, j * 128:(j + 1) * 128], ident)
                        sc = t * (TCH // 128) + j
                        nc.scalar.activation(
                            out=V[:, sc, :, 0:DH],
                            in_=pvt.rearrange("p (h d) -> p h d", h=NH),
                            func=AF.Copy, scale=rv_all[:, sc:sc + 1])

                def u_evict(ps, g):
                    tmp_u = work.tile([128, TCH], F32, tag="tmp_u")
                    nc.vector.tensor_tensor(out=tmp_u, in0=ps, in1=r_rep,
                                            op=OP.mult)
                    nc.scalar.activation(out=U[:, g, ts0:ts1], in_=tmp_u,
                                         func=AF.Relu)

                for gg in range(7):
                    ps = mm_ps.tile([128, TCH], F32, tag="mm")
                    if gg < 3:
                        w_sb, aug_sb, gs = wqkv_sb, aug_qkv, slice(gg * 128, (gg + 1) * 128)
                        ev = qt_evict if gg == 0 else (kt_evict if gg == 1 else v_evict)
                    else:
                        g = gg - 3
                        w_sb, aug_sb, gs = w1_sb, aug_w1, slice(g * 128, (g + 1) * 128)
                        ev = (lambda ps, g=g: u_evict(ps, g))
                    for c in range(EK):
                        nc.tensor.matmul(ps, w_sb[:, c, gs], xs[:, c, :],
                                         start=(c == 0), stop=False)
                    nc.tensor.matmul(ps, aug_sb[0:33, gs], aug[0:33, :],
                                     start=False, stop=True)
                    ev(ps)

            for t in range(NCH + LAG):
                if t >= LAG:
                    proj_part(t - LAG)
                if t < NCH:
                    stats_part(t)

        # deferred CD-only loads (transfer during the projection phase)
        nc.gpsimd.dma_start(out=w2_sb, in_=w2.rearrange("(k p) e -> p k e", p=128))
        nc.gpsimd.dma_start(out=wo_sb, in_=wo[:, :])

        # ---------------- Phase CD: attention + output, per (batch, t-chunk) --
        with ExitStack() as cd, \
             tc.tile_pool(name="expp", bufs=4) as expp, \
             tc.tile_pool(name="cdwork", bufs=2) as cdw, \
             tc.tile_pool(name="zp", bufs=2) as zp, \
             tc.tile_pool(name="s_ps", bufs=2, space="PSUM") as s_ps, \
             tc.tile_pool(name="o_ps", bufs=1, space="PSUM") as o_ps, \
             tc.tile_pool(name="y_ps", bufs=2, space="PSUM") as y_ps:
            nsc = T // SC

            def out_group(ts0, ts1, g):
                # one output tile: y^T[gs, ts] = W2^T u + Wo^T O^T (PSUM-accum)
                ps_y = y_ps.tile([128, TCH], F32, tag="y", name="ps_y")
                gs = slice(g * 128, (g + 1) * 128)
                for k in range(FFC // 128):
                    nc.tensor.matmul(ps_y, w2_sb[:, k, gs], U[:, k, ts0:ts1],
                                     start=(k == 0), stop=False)
                nc.tensor.matmul(ps_y, wo_sb[:, gs], OT[:, ts0:ts1],
                                 start=False, stop=True)
                y_sb = cdw.tile([128, TCH], BF16, tag="y_sb", name="y_sb")
                nc.vector.tensor_copy(out=y_sb, in_=ps_y)
                nc.sync.dma_start(out=yTr[g, :, ts0:ts1], in_=y_sb)

            pending = []  # deferred OUT groups of the previous iteration
            for b in range(B):
                for tq in range(T // TCH):
                    ts0 = b * T + tq * TCH
                    ts1 = ts0 + TCH

                    ps_o = [o_ps.tile([DH + 1, TCH], F32, tag=f"o{h}", name=f"ps_o{h}")
                            for h in range(NH)]
                    exs = {}
                    # scores+exp run one wave ahead of PV; previous iter's OUT
                    # groups interleave so the PE stays dense during exp
                    for sc in range(nsc + 4):
                        if sc < nsc:
                            gsc = b * nsc + sc
                            ps_s = s_ps.tile([128, NH, TCH], F32, tag="s", name="ps_s")
                            for h in range(NH):
                                hs = slice(h * DH, (h + 1) * DH)
                                # row-tiled pair: head0 at PE rows 0-63,
                                # head1 at rows 64-127 run concurrently
                                nc.tensor.matmul(
                                    ps_s[:, h, :], KT[hs, gsc * SC:(gsc + 1) * SC],
                                    QT[hs, ts0:ts1], start=True, stop=True)
                            ex = expp.tile([128, NH, TCH], BF16, tag="ex", name="ex",
                                           bufs=6)
                            nc.scalar.activation(out=ex, in_=ps_s, func=AF.Exp,
                                                 scale=r8_all[:, gsc:gsc + 1])
                            exs[sc] = ex
                            if dbg and b == 0 and tq == 0 and sc == 0:
                                s_sb = cdw.tile([128, NH, TCH], F32, tag="dbgs")
                                nc.vector.tensor_copy(out=s_sb, in_=ps_s)
                                nc.sync.dma_start(
                                    out=dbg_s[:, :],
                                    in_=s_sb.rearrange("p a b -> p (a b)"))
                                nc.sync.dma_start(
                                    out=dbg_ex[:, :],
                                    in_=ex.rearrange("p a b -> p (a b)"))
                        if sc >= 4:
                            psc = sc - 4
                            gpsc = b * nsc + psc
                            ex = exs.pop(psc)
                            for h in range(NH):
                                nc.tensor.matmul(ps_o[h], V[:, gpsc, h, :],
                                                 ex[:, h, :],
                                                 start=(psc == 0),
                                                 stop=(psc == nsc - 1))
                        if pending and sc >= 6:
                            pending.pop(0)()

                    while pending:
                        pending.pop(0)()

                    # evict both accumulators, then normalize via exp(-ln Z)
                    ous = []
                    for h in range(NH):
                        ou = cdw.tile([DH + 1, TCH], F32, tag=f"ou{h}", name="ou")
                        nc.vector.tensor_copy(out=ou, in_=ps_o[h])
                        ous.append(ou)
                    for h in range(NH):
                        ou = ous[h]
                        if dbg and b == 0 and tq == 0:
                            nc.sync.dma_start(
                                out=dbg_ou[:, h * TCH:(h + 1) * TCH], in_=ou)
                        zrow = zp.tile([1, TCH], F32, tag="zrow", name="zrow")
                        nc.vector.tensor_copy(out=zrow, in_=ou[DH:DH + 1, :])
                        zinv = zp.tile([1, TCH], F32, tag="zinv", name="zinv")
                        nc.vector.reciprocal_approx_fast(out=zinv, in_=zrow)
                        zb = dpool.tile([1, TCH], F32, tag="zb", name="zb")
                        nc.gpsimd.dma_start(out=zb, in_=zinv)
                        if dbg and b == 0 and tq == 0:
                            nc.sync.dma_start(
                                out=dbg_zi[0:1, h * TCH:(h + 1) * TCH], in_=zinv)
                        zrep = zp.tile([DH, TCH], F32, tag="zrep", name="zrep")
                        nc.gpsimd.dma_start(out=zrep, in_=_bcast(zb[0:1, :], DH))
                        nc.vector.tensor_tensor(
                            out=OT[h * DH:(h + 1) * DH, ts0:ts1],
                            in0=ou[0:DH, :], in1=zrep, op=OP.mult)

                    pending = [
                        (lambda g=g, a=ts0, z=ts1: out_group(a, z, g))
                        for g in range(EK)]

            while pending:
                pending.pop(0)()

            if dbg:
                nc.sync.dma_start(out=dbg_qt[:, :], in_=QT)
                nc.sync.dma_start(out=dbg_kt[:, :], in_=KT)
                nc.sync.dma_start(out=dbg_ot[:, :], in_=OT)
                nc.sync.dma_start(out=dbg_v[:, :], in_=V.rearrange("p a b o -> p (a b o)"))
                nc.sync.dma_start(out=dbg_u[:, :], in_=U.rearrange("p a t -> p (a t)"))

    nc.finalize()
    return nc


_CACHE = {}


def _get_program():
    if "nc" not in _CACHE:
        _CACHE["nc"] = _build_device_program()
    return _CACHE["nc"]


def _bf(a):
    return np.ascontiguousarray(np.asarray(a, np.float32)).astype(ml_dtypes.bfloat16)


def _host_prepare(x, Wq, Wk, Wv, Wo, bo, W1, b1, W2, b2, g1, be1, g2, be2):
    xf = np.ascontiguousarray(np.asarray(x, np.float32).reshape(TN, E))
    xT = _bf(xf.T)
    Wq, Wk, Wv = (np.asarray(w, np.float32) for w in (Wq, Wk, Wv))
    Wo, W1, W2 = (np.asarray(w, np.float32) for w in (Wo, W1, W2))
    g1, be1, g2, be2 = (np.asarray(v, np.float32) for v in (g1, be1, g2, be2))
    b1 = np.asarray(b1, np.float32)

    in_maps = []
    for c in range(NCORES):
        hs = [NH * c + i for i in range(NH)]

        def qkv_block(W):
            Wc = W[hs]                                   # [NH, E, DH]
            Wp = (g1[None, :, None] * Wc)                # diag(g1) @ W
            main = np.transpose(Wp, (1, 0, 2)).reshape(E, NH * DH)
            A = np.einsum("e,hed->hd", g1, Wc).reshape(NH * DH)
            C = np.einsum("e,hed->hd", be1, Wc).reshape(NH * DH)
            return np.concatenate([main, -A[None, :], C[None, :]], axis=0)

        wqkv = np.concatenate([qkv_block(Wq), qkv_block(Wk), qkv_block(Wv)], axis=1)

        J = slice(FFC * c, FFC * (c + 1))
        W1c = W1[:, J]
        w1_main = g2[:, None] * W1c
        A1 = g2 @ W1c
        C1 = be2 @ W1c + b1[J]
        w1m = np.concatenate([w1_main, -A1[None, :], C1[None, :]], axis=0)

        in_maps.append({
            "xT": xT,
            "wqkv": _bf(wqkv),
            "w1": _bf(w1m),
            "w2": _bf(W2[J, :]),
            "wo": _bf(Wo[NH * DH * c: NH * DH * (c + 1), :]),
            "ident": _bf(np.eye(128, dtype=np.float32)),
            "ones1": _bf(np.ones((128, 1), np.float32)),
        })
    return xf, in_maps


def _host_finish(x, bo, b2, xf, results):
    acc = xf.copy()
    for res in results:
        acc += np.asarray(res["yT"], np.float32).T
    acc += np.asarray(bo, np.float32)[None, :] + np.asarray(b2, np.float32)[None, :]
    return acc.reshape(np.asarray(x).shape).astype(np.float32)


def kernel(x, Wq, Wk, Wv, Wo, bo, W1, b1, W2, b2, g1, be1, g2, be2, _trace=False):
    nc = _get_program()
    xf, in_maps = _host_prepare(x, Wq, Wk, Wv, Wo, bo, W1, b1, W2, b2, g1, be1, g2, be2)
    out = run_bass_kernel_spmd(nc, in_maps, list(range(NCORES)), trace=_trace)
    result = _host_finish(x, bo, b2, xf, out.results)
    if _trace:
        return result, out
    return result
